# revision 1
# baseline (speedup 1.0000x reference)
"""Trainium2 Bass kernel for nn_Basic_Block_v1 (spatial/spectral Mamba2 block).

Sharding: data-parallel over batch (16 samples) across 8 NeuronCores,
2 samples per core; all parameters replicated. The SSD scans are computed
in closed quadratic form (masked decay matrix x dt-scaled inputs) so all
heavy math runs on the TensorEngine.
"""
import sys
sys.path.insert(0, '/opt/trn_rl_repo')
import json
import os

import numpy as np

import concourse.bass as bass
import concourse.mybir as mybir
from concourse import tile
from concourse.bass_utils import run_bass_kernel_spmd

F32 = mybir.dt.float32
I32 = mybir.dt.int32
AF = mybir.ActivationFunctionType
ALU = mybir.AluOpType
AX = mybir.AxisListType

NCORES = 8
BPC = 2          # batch per core
L = 256          # spatial tokens
C = 128          # channels
H1 = 4           # spa heads
DI1 = 256        # spa d_inner
H2 = 8           # spe heads
DI2 = 512        # spe d_inner
L2 = 128         # spe tokens (channels)
DM2 = 256        # spe d_model (seq positions)
NST = 64         # d_state
EPS = 1e-5

# ---------------------------------------------------------------------------
# walrus in this container supports only ONE sync-wait per instruction;
# split extra waits emitted by the Tile scheduler onto preceding NoOps.
_WAIT_LIMIT = 1
_orig_to_json = bass.Bass.to_json_bytes


def _fix_block(b, ctr):
    insts = b.get('instructions')
    if insts:
        out = []
        for ins in insts:
            si = ins.get('sync_info')
            waits = (si or {}).get('on_wait') or []
            if len(waits) > _WAIT_LIMIT:
                while len(waits) > _WAIT_LIMIT:
                    chunk, waits = waits[:_WAIT_LIMIT], waits[_WAIT_LIMIT:]
                    ctr[0] += 1
                    out.append({
                        "debug": ins.get("debug"),
                        "engine": ins["engine"],
                        "ins": [],
                        "name": f"I-wsplit{ctr[0]}",
                        "opcode": "NoOp",
                        "outs": [],
                        "text_hint": "wsplit",
                        "sync_info": {"on_update": [], "on_wait": chunk},
                    })
                si['on_wait'] = waits
            out.append(ins)
        b['instructions'] = out
    for sb in b.get('blocks') or []:
        _fix_block(sb, ctr)


def _patched_to_json(self, *a, **k):
    raw = _orig_to_json(self, *a, **k)
    d = json.loads(raw)
    ctr = [0]
    for f in d.get('functions', []):
        for b in f.get('blocks', []):
            _fix_block(b, ctr)
    if ctr[0] == 0:
        return raw
    return json.dumps(d).encode()


bass.Bass.to_json_bytes = _patched_to_json


# ---------------------------------------------------------------------------
def _sincos_2d(dim, Hg):
    def e1(d, pos):
        omega = 1.0 / (10000.0 ** (np.arange(d // 2, dtype=np.float64) / (d / 2.0)))
        out = pos[:, None] * omega[None, :]
        return np.concatenate([np.sin(out), np.cos(out)], axis=-1)
    gh, gw = np.meshgrid(np.arange(Hg), np.arange(Hg), indexing='ij')
    emb = np.concatenate([e1(dim // 2, gh.reshape(-1)), e1(dim // 2, gw.reshape(-1))], axis=-1)
    return emb.astype(np.float32)


def host_constants():
    d = {}
    d['pe_fm'] = np.ascontiguousarray(_sincos_2d(C, 16).T)              # [128, 256]
    d['ident'] = np.eye(128, dtype=np.float32)
    iota = np.arange(L, dtype=np.float32)
    d['iotaC'] = np.stack([iota[:128], iota[128:]], axis=1).copy()      # [128, 2]
    # maskT[st][sp][t] = 1 if (st*128+sp) <= t   (spa, L=256)
    sidx = np.arange(L)[:, None]
    tidx = np.arange(L)[None, :]
    m = (sidx <= tidx).astype(np.float32)                               # [s, t]
    d['maskT_spa'] = np.stack([m[:128], m[128:]], axis=1).copy()        # [128, 2, 256]
    s2 = np.arange(L2)[:, None]
    t2 = np.arange(L2)[None, :]
    d['maskT_spe'] = (s2 <= t2).astype(np.float32)                      # [128, 128]
    # head one-hots for dt broadcast: E[k, j, m] = 1 iff k == 2j + m//64
    E1 = np.zeros((H1, 2, 128), np.float32)
    for j in range(2):
        for m in range(128):
            E1[2 * j + m // 64, j, m] = 1.0
    d['E_spaJ'] = E1
    E2 = np.zeros((H2, 4, 128), np.float32)
    for j in range(4):
        for m in range(128):
            E2[2 * j + m // 64, j, m] = 1.0
    d['E_speJ'] = E2
    EA = np.zeros((8, 128), np.float32)
    for h in range(8):
        EA[h, h * 16:(h + 1) * 16] = 1.0
    d['E_attn'] = EA                                                    # [8, 128]
    d['Emask_q'] = EA.T.copy()                                          # [128, 8]
    d['onesrow'] = np.ones(512, np.float32)
    return d


COL_ORDER = (
    ["spa_dtb0", "spa_alog0", "spa_cb0_0", "spa_cb0_1", "spa_cbB0", "spa_cbC0",
     "spa_dpc0_0", "spa_dpc0_1", "spa_rwc0_0", "spa_rwc0_1",
     "spa_dtb1", "spa_alog1", "spa_cb1_0", "spa_cb1_1", "spa_cbB1", "spa_cbC1",
     "spa_dpc1_0", "spa_dpc1_1", "spa_rwc1_0", "spa_rwc1_1"]
    + ["spe_dtb0", "spe_alog0", "spe_cb0_0", "spe_cb0_1", "spe_cb0_2", "spe_cb0_3",
       "spe_cbB0", "spe_cbC0",
       "spe_dpc0_0", "spe_dpc0_1", "spe_dpc0_2", "spe_dpc0_3",
       "spe_rwc0_0", "spe_rwc0_1", "spe_rwc0_2", "spe_rwc0_3",
       "spe_dtb1", "spe_alog1", "spe_cb1_0", "spe_cb1_1", "spe_cb1_2", "spe_cb1_3",
       "spe_cbB1", "spe_cbC1",
       "spe_dpc1_0", "spe_dpc1_1", "spe_dpc1_2", "spe_dpc1_3",
       "spe_rwc1_0", "spe_rwc1_1", "spe_rwc1_2", "spe_rwc1_3"]
    + ["lnw_spa0", "lnw_spa1", "lnw_norm",
       "cprj_b", "aq_b", "ak_b", "av_b", "ao_b",
       "sq_b0", "sq_b1", "sk_b0", "sk_b1"]
)
CIDX = {k: ix for ix, k in enumerate(COL_ORDER)}


def prep_weights(inp):
    """Host-side layout prep of the replicated parameters (tile layouts,
    single DMA per tensor)."""
    w = {}
    w['spa_in_wT'] = np.ascontiguousarray(np.transpose(inp['spa_in_w'], (0, 2, 1)))
    cv = np.zeros((128, 2, 4, 4), np.float32)
    for i in range(2):
        cv[:, i, 0] = inp['spa_conv_w'][i, 0:128]
        cv[:, i, 1] = inp['spa_conv_w'][i, 128:256]
        cv[0:64, i, 2] = inp['spa_conv_w'][i, 256:320]
        cv[0:64, i, 3] = inp['spa_conv_w'][i, 320:384]
    w['spa_conv_pk'] = cv
    sow = np.transpose(inp['spa_out_w'], (0, 2, 1)).reshape(2, 2, 128, 128)
    w['spa_out_pk'] = np.ascontiguousarray(sow.transpose(2, 0, 1, 3))
    w['spe_ln_wB'] = np.ascontiguousarray(np.broadcast_to(
        inp['spe_ln_w'][:, None, :], (2, 128, 256)).transpose(1, 0, 2))
    w['spe_ln_bB'] = np.ascontiguousarray(np.broadcast_to(
        inp['spe_ln_b'][:, None, :], (2, 128, 256)).transpose(1, 0, 2))
    siw = np.transpose(inp['spe_in_w'], (0, 2, 1)).reshape(2, 2, 128, 1160)
    w['spe_in_pk'] = np.ascontiguousarray(siw.transpose(0, 2, 1, 3))
    cv2 = np.zeros((128, 2, 6, 4), np.float32)
    for i in range(2):
        for j in range(4):
            cv2[:, i, j] = inp['spe_conv_w'][i, j * 128:(j + 1) * 128]
        cv2[0:64, i, 4] = inp['spe_conv_w'][i, 512:576]
        cv2[0:64, i, 5] = inp['spe_conv_w'][i, 576:640]
    w['spe_conv_pk'] = cv2
    sew = np.transpose(inp['spe_out_w'], (0, 2, 1)).reshape(2, 4, 128, 256)
    w['spe_out_pk'] = np.ascontiguousarray(sew.transpose(0, 2, 1, 3))
    w['cprj_pk'] = np.ascontiguousarray(
        np.transpose(inp['cprj_w'], (2, 1, 0)).transpose(1, 0, 2))
    for nm in ('aq', 'ak', 'av', 'ao'):
        w[nm + 'T'] = np.ascontiguousarray(inp[nm + '_w'].T)
    for nm in ('sq', 'sk', 'sv', 'so'):
        wt_ = inp[nm + '_w'].T.reshape(2, 128, 256)
        w[nm + 'T'] = np.ascontiguousarray(wt_.transpose(1, 0, 2))
    w['svbB'] = np.ascontiguousarray(np.broadcast_to(inp['sv_b'][None, :], (128, 256)))
    w['sobB'] = np.ascontiguousarray(np.broadcast_to(inp['so_b'][None, :], (128, 256)))
    w['dsw_pk'] = np.ascontiguousarray(
        inp['ds_conv_w'].reshape(9, 128, 128).transpose(1, 0, 2))
    w['ds_ln_wB'] = np.ascontiguousarray(np.broadcast_to(inp['ds_ln_w'][None, :], (64, 128)))
    w['ds_ln_bB'] = np.ascontiguousarray(np.broadcast_to(inp['ds_ln_b'][None, :], (64, 128)))
    lnwb = np.zeros((2, 3, 128), np.float32)
    lnwb[0, 0], lnwb[1, 0] = inp['spa_ln_w'][0], inp['spa_ln_b'][0]
    lnwb[0, 1], lnwb[1, 1] = inp['spa_ln_w'][1], inp['spa_ln_b'][1]
    lnwb[0, 2], lnwb[1, 2] = inp['norm_w'], inp['norm_b']
    w['lnwb'] = lnwb
    cols = {}
    for i in range(2):
        cols[f"spa_dtb{i}"] = inp['spa_dt_bias'][i]
        cols[f"spa_alog{i}"] = inp['spa_A_log'][i]
        cols[f"spa_cb{i}_0"] = inp['spa_conv_b'][i, 0:128]
        cols[f"spa_cb{i}_1"] = inp['spa_conv_b'][i, 128:256]
        cols[f"spa_cbB{i}"] = inp['spa_conv_b'][i, 256:320]
        cols[f"spa_cbC{i}"] = inp['spa_conv_b'][i, 320:384]
        for j in range(2):
            cols[f"spa_dpc{i}_{j}"] = np.repeat(inp['spa_D'][i], 64)[j * 128:(j + 1) * 128]
            cols[f"spa_rwc{i}_{j}"] = inp['spa_rms_w'][i, j * 128:(j + 1) * 128]
        cols[f"spe_dtb{i}"] = inp['spe_dt_bias'][i]
        cols[f"spe_alog{i}"] = inp['spe_A_log'][i]
        for j in range(4):
            cols[f"spe_cb{i}_{j}"] = inp['spe_conv_b'][i, j * 128:(j + 1) * 128]
            cols[f"spe_dpc{i}_{j}"] = np.repeat(inp['spe_D'][i], 64)[j * 128:(j + 1) * 128]
            cols[f"spe_rwc{i}_{j}"] = inp['spe_rms_w'][i, j * 128:(j + 1) * 128]
        cols[f"spe_cbB{i}"] = inp['spe_conv_b'][i, 512:576]
        cols[f"spe_cbC{i}"] = inp['spe_conv_b'][i, 576:640]
    cols["lnw_spa0"] = inp['spa_ln_w'][0]
    cols["lnw_spa1"] = inp['spa_ln_w'][1]
    cols["lnw_norm"] = inp['norm_w']
    cols["cprj_b"] = inp['cprj_b']
    for nm in ('aq', 'ak', 'av', 'ao'):
        cols[nm + "_b"] = inp[nm + '_b']
    cols["sq_b0"] = inp['sq_b'][0:128]
    cols["sq_b1"] = inp['sq_b'][128:256]
    cols["sk_b0"] = inp['sk_b'][0:128]
    cols["sk_b1"] = inp['sk_b'][128:256]
    pk = np.zeros((128, len(COL_ORDER)), np.float32)
    for k, v in cols.items():
        v = np.asarray(v, np.float32)
        pk[0:v.shape[0], CIDX[k]] = v
    w['colpak'] = pk
    return w



# ---------------------------------------------------------------------------
def build_program(taps=()):
    """Builds the per-core SPMD Bass program. `taps` is a set of intermediate
    names to also write to DRAM outputs (debug only)."""
    nc = bass.Bass()

    def din(name, shape, dt=F32):
        return nc.dram_tensor(name, shape, dt, kind="ExternalInput")

    x2 = din("x2", [BPC, C, L])
    idx = din("idx", [BPC, L], I32)
    inv = din("inv", [BPC, L], I32)

    cst = host_constants()
    cst_t = {k: din(k, list(v.shape)) for k, v in cst.items()}

    wnames = {
        'spa_in_wT': [2, 128, 644], 'spa_conv_pk': [128, 2, 4, 4],
        'spa_out_pk': [128, 2, 2, 128],
        'spe_ln_wB': [128, 2, 256], 'spe_ln_bB': [128, 2, 256],
        'spe_in_pk': [2, 128, 2, 1160], 'spe_conv_pk': [128, 2, 6, 4],
        'spe_out_pk': [2, 128, 4, 256],
        'cprj_pk': [128, 5, 128],
        'aqT': [128, 128], 'akT': [128, 128], 'avT': [128, 128], 'aoT': [128, 128],
        'sqT': [128, 2, 256], 'skT': [128, 2, 256], 'svT': [128, 2, 256],
        'soT': [128, 2, 256], 'svbB': [128, 256], 'sobB': [128, 256],
        'dsw_pk': [128, 9, 128], 'ds_ln_wB': [64, 128], 'ds_ln_bB': [64, 128],
        'lnwb': [2, 3, 128], 'colpak': [128, len(COL_ORDER)],
    }
    w_t = {k: din(k, shp) for k, shp in wnames.items()}

    out = nc.dram_tensor("out", [BPC, 8, 8, C], F32, kind="ExternalOutput")
    tap_t = {}

    with tile.TileContext(nc) as tc:
        import contextlib
        stk = contextlib.ExitStack()
        sb = stk.enter_context(tc.tile_pool(name="sb", bufs=1))
        ps1 = stk.enter_context(tc.tile_pool(name="ps1", bufs=2, space="PSUM"))
        ps2 = stk.enter_context(tc.tile_pool(name="ps2", bufs=3, space="PSUM"))
        psS = stk.enter_context(tc.tile_pool(name="psS", bufs=2, space="PSUM"))
        psD = stk.enter_context(tc.tile_pool(name="psD", bufs=1, space="PSUM"))

        BUFS2 = {"cv_a0", "cv_a1", "rowA", "rowB", "tm_tmp", "ssd_Dt"}

        def T(shape, tag, dt=F32):
            return sb.tile(shape, dt, tag=tag, name=tag,
                           bufs=2 if tag in BUFS2 else 1)

        def P512(tag="b512"):
            return ps1.tile([128, 512], F32, tag=tag, name=tag)

        def P256(tag="b256"):
            return ps2.tile([128, 256], F32, tag=tag, name=tag)

        def tap(name, ap_fn):
            # ap_fn: callable giving (dram_shape, writer) – writer(dram) DMAs data
            if name in taps:
                shape, writer = ap_fn()
                t = nc.dram_tensor("t_" + name, shape, F32, kind="ExternalOutput")
                tap_t[name] = t
                writer(t)

        dma = nc.sync.dma_start
        V = nc.vector
        S = nc.scalar

        # ---------- load constants ----------
        ct = {}
        for k, v in cst.items():
            if k == 'onesrow':
                continue
            ct[k] = T(list(v.shape), "c_" + k)
            dma(ct[k][:], cst_t[k][:])

        # ---------- preload weights (single DMA per tensor) ----------
        wt = {}
        for name, shp in wnames.items():
            if name in ('spa_in_wT', 'spe_in_pk', 'spe_out_pk'):
                continue
            t = T(shp, "w_" + name)
            dma(t[:], w_t[name][:])
            wt[name] = t
        colpak = wt['colpak']

        def col(key, p=128):
            return colpak[0:p, CIDX[key]:CIDX[key] + 1]

        ones4 = T([128, 128], "ones4")
        V.memset(ones4[:], 1.0)
        epscol = T([128, 1], "epscol")
        V.memset(epscol[:], EPS)
        onescol = ones4[:, 0:1]       # [128,1]
        onesrow1 = ones4[0:1, :]      # [1,128]
        ident = ct['ident']

        # ---------- stage 0: embed + permute ----------
        xb = T([128, BPC, L], "xb")
        for s in range(BPC):
            dma(xb[:, s, :], x2[s])
        x0 = T([128, BPC, L], "x0")
        V.tensor_tensor(
            x0[:], xb[:],
            ct['pe_fm'][:].unsqueeze(1).to_broadcast((128, BPC, L)),
            op=ALU.add)

        idxr = T([1, BPC, L], "irow_raw", I32)
        dma(idxr[:], idx[None, :, :])
        idxf = T([1, BPC, L], "irow_f")
        V.tensor_copy(idxf[:], idxr[:])

        xs = T([128, BPC, L], "xs")
        for s in range(BPC):
            # PmT[st][sp][t] = (idx[t] == st*128+sp)
            idxB = P512()
            nc.tensor.matmul(idxB[:, 0:L], onesrow1, idxf[:, s, :], start=True, stop=True)
            PmT = T([128, 2, L], "perm_oh")
            for st in range(2):
                V.tensor_scalar(PmT[:, st, :], idxB[:, 0:L], ct['iotaC'][:, st:st + 1],
                                None, op0=ALU.is_equal)
            # x0 token-major
            x0tm = T([128, 2, 128], "tm_tmp")
            for tt in range(2):
                ptr = P256()
                nc.tensor.transpose(ptr[:, 0:128], x0[:, s, tt * 128:(tt + 1) * 128], ident[:])
                S.copy(x0tm[:, tt, :], ptr[:, 0:128])
            pxs = P256()
            for st in range(2):
                nc.tensor.matmul(pxs[:], x0tm[:, st, :], PmT[:, st, :],
                                 start=(st == 0), stop=(st == 1))
            S.copy(xs[:, s, :], pxs[:])

        def tap_batched(t_sb, shape_per_s):
            def writer(dram):
                for s in range(BPC):
                    dma(dram[s], t_sb[:, s, :])
            return ([BPC] + shape_per_s, writer)

        tap("xs0", lambda: tap_batched(xs, [128, L]))

        # ================= shared helpers =================
        lnrhs = T([2, 512], "ln_rhs")
        dma(lnrhs[1:2, :], cst_t['onesrow'][None, :])

        def part_ln(xflat, lnidx):
            """LayerNorm over the channel (partition) dim of [128, 512]."""
            sq = T([128, 512], "sq_tmp")
            S.activation(sq[:], xflat, AF.Square)
            msum = psS.tile([1, 512], F32, tag="small", name="small")
            nc.tensor.matmul(msum[:], onescol, xflat, start=True, stop=True)
            murow = T([1, 512], "ln_mu")
            V.tensor_scalar(murow[:], msum[:], 1.0 / 128, None, op0=ALU.mult)
            ssum = psS.tile([1, 512], F32, tag="small", name="small")
            nc.tensor.matmul(ssum[:], onescol, sq[:], start=True, stop=True)
            mu2 = T([1, 512], "rowA")
            V.tensor_mul(mu2[:], murow[:], murow[:])
            var = T([1, 512], "rowB")
            V.scalar_tensor_tensor(var[:], ssum[:], 1.0 / 128, mu2[:],
                                   op0=ALU.mult, op1=ALU.subtract)
            lnv = T([1, 512], "rowA")
            S.activation(lnv[:], var[:], AF.Ln, bias=epscol[0:1, 0:1])
            rstd = T([1, 512], "ln_rstd")
            S.activation(rstd[:], lnv[:], AF.Exp, scale=-0.5)
            V.scalar_tensor_tensor(lnrhs[0:1, :], murow[:], -1.0, rstd[:],
                                   op0=ALU.mult, op1=ALU.mult)
            Rp = P512()
            nc.tensor.matmul(Rp[:], wt['lnwb'][:, lnidx, :], lnrhs[:],
                             start=True, stop=True)
            rstdB = P512()
            nc.tensor.matmul(rstdB[:], onesrow1, rstd[:], start=True, stop=True)
            wcol = col(("lnw_spa0", "lnw_spa1", "lnw_norm")[lnidx])
            tmp = T([128, 512], "ln_tmp")
            V.tensor_mul(tmp[:], xflat, rstdB[:])
            xln = T([128, 512], "ln_out")
            V.scalar_tensor_tensor(xln[:], tmp[:], wcol, Rp[:],
                                   op0=ALU.mult, op1=ALU.add)
            return xln

        def convchain(buf, wc, cb, P, W, tag):
            """Causal depthwise conv (k=4) + silu. buf [P, 2, W+3]; returns [P, 2, W]."""
            a0 = T([P, 2, W], "cv_a0")
            V.tensor_scalar(a0[:], buf[:, :, 0:W], wc[:, 0:1], None, op0=ALU.mult)
            a1 = T([P, 2, W], "cv_a1")
            V.scalar_tensor_tensor(a1[:], buf[:, :, 1:W + 1], wc[:, 1:2], a0[:],
                                   op0=ALU.mult, op1=ALU.add)
            a2 = T([P, 2, W], "cv_a0")
            V.scalar_tensor_tensor(a2[:], buf[:, :, 2:W + 2], wc[:, 2:3], a1[:],
                                   op0=ALU.mult, op1=ALU.add)
            a3 = T([P, 2, W], "cv_a1")
            V.scalar_tensor_tensor(a3[:], buf[:, :, 3:W + 3], wc[:, 3:4], a2[:],
                                   op0=ALU.mult, op1=ALU.add)
            xc = T([P, 2, W], tag)
            S.activation(xc[:], a3[:], AF.Silu, bias=cb[:, 0:1])
            return xc

        # ================= spa mamba =================
        def spa_mamba(i, xs):
            xflat = xs[:].rearrange("p s t -> p (s t)")
            xln = part_ln(xflat, i)
            tap(f"xln{i}", lambda: ([128, 512], lambda d: dma(d[:], xln[:])))
            inw_t = T([128, 644], "w_spa_in")
            dma(inw_t[:], w_t['spa_in_wT'][i])
            inw = inw_t[:]
            # in_proj: z (2 blocks), x (2 blocks), B, C, dt
            zsb = T([128, 2, 512], "mb_z")
            for j in range(2):
                pz = P512()
                nc.tensor.matmul(pz[:], inw[:, j * 128:(j + 1) * 128], xln[:],
                                 start=True, stop=True)
                S.copy(zsb[:, j, :], pz[:])
            cvx = []
            for j in range(2):
                px = P512()
                nc.tensor.matmul(px[:], inw[:, 256 + j * 128:256 + (j + 1) * 128], xln[:],
                                 start=True, stop=True)
                buf = T([128, 2, 259], f"cv_x{j}")
                V.memset(buf[:, :, 0:3], 0.0)
                S.copy(buf[:, :, 3:259], px[:].rearrange("p (s t) -> p s t", s=2))
                cvx.append(buf)
            cvbc = []
            for nm, off in (("B", 512), ("C", 576)):
                pb = P512()
                nc.tensor.matmul(pb[0:64, :], inw[:, off:off + 64], xln[:],
                                 start=True, stop=True)
                buf = T([64, 2, 259], "cv_" + nm)
                V.memset(buf[:, :, 0:3], 0.0)
                S.copy(buf[:, :, 3:259], pb[0:64, :].rearrange("p (s t) -> p s t", s=2))
                cvbc.append(buf)
            pdt = psS.tile([4, 512], F32, tag="small", name="small")
            nc.tensor.matmul(pdt[:], inw[:, 640:644], xln[:], start=True, stop=True)
            # softplus(dt + bias) = ln(1 + exp(dt + bias))
            e1 = T([4, 512], "rowA")
            S.activation(e1[:], pdt[:], AF.Exp, bias=col(f"spa_dtb{i}", 4))
            e1p = T([4, 512], "rowB")
            V.tensor_scalar(e1p[:], e1[:], 1.0, None, op0=ALU.add)
            dtv = T([4, 512], "mb_dtv")
            S.activation(dtv[:], e1p[:], AF.Ln)
            eA = T([4, 1], "spa_eA")
            S.activation(eA[:], col(f"spa_alog{i}", 4), AF.Exp)
            dtA = T([4, 512], "rowA")
            V.tensor_scalar(dtA[:], dtv[:], eA[:, 0:1], -1.0, op0=ALU.mult, op1=ALU.mult)
            acum = T([4, 512], "mb_acum")
            aflat = T([1, 2, 1024], "aflat")
            for s in range(BPC):
                V.tensor_tensor_scan(acum[:, s * 256:(s + 1) * 256],
                                     dtA[:, s * 256:(s + 1) * 256],
                                     dtA[:, s * 256:(s + 1) * 256], 0.0,
                                     op0=ALU.add, op1=ALU.bypass)
                dma(aflat[0:1, s, :].rearrange("o (p f) -> o p f", p=4),
                    acum[:, s * 256:(s + 1) * 256])
            tap(f"dtv{i}", lambda: ([4, 512], lambda d: dma(d[:], dtv[:])))
            tap(f"acum{i}", lambda: ([4, 512], lambda d: dma(d[:], acum[:])))
            # conv + silu
            xc = []
            for j in range(2):
                xc.append(convchain(cvx[j], wt['spa_conv_pk'][:, i, j, :],
                                    col(f"spa_cb{i}_{j}"), 128, 256, f"xc_{j}"))
            xcB = convchain(cvbc[0], wt['spa_conv_pk'][0:64, i, 2, :],
                            col(f"spa_cbB{i}", 64), 64, 256, "xc_B")
            xcC = convchain(cvbc[1], wt['spa_conv_pk'][0:64, i, 3, :],
                            col(f"spa_cbC{i}", 64), 64, 256, "xc_C")
            if i == 0:
                tap("xc00", lambda: ([128, 512], lambda d: dma(
                    d[:], xc[0][:].rearrange("p s t -> p (s t)"))))
                tap("xcB0", lambda: ([64, 512], lambda d: dma(
                    d[:], xcB[:].rearrange("p s t -> p (s t)"))))
                tap("xcC0", lambda: ([64, 512], lambda d: dma(
                    d[:], xcC[:].rearrange("p s t -> p (s t)"))))
            # z silu
            zsil = T([128, 2, 512], "mb_zsil")
            for j in range(2):
                S.activation(zsil[:, j, :], zsb[:, j, :], AF.Silu)
            # dt-scaled x (feature-major): xp[:, j, :] = xc[j] * dtB_j
            xp = T([128, 2, 512], "mb_xp")
            for j in range(2):
                pdb = P512()
                nc.tensor.matmul(pdb[:], ct['E_spaJ'][:, j, :], dtv[:], start=True, stop=True)
                V.tensor_mul(xp[:, j, :], xc[j][:].rearrange("p s t -> p (s t)"), pdb[:])
            h1 = T([128, 2, 256], "h1")
            for s in range(BPC):
                # token-major dt-scaled x: xtm [t(128), st, hp(256)]
                xtm = T([128, 2, 256], "spa_xtm")
                for st in range(2):
                    for j in range(2):
                        ptr = P256()
                        nc.tensor.transpose(
                            ptr[:, 0:128],
                            xp[:, j, s * 256 + st * 128: s * 256 + (st + 1) * 128],
                            ident[:])
                        S.copy(xtm[:, st, j * 128:(j + 1) * 128], ptr[:, 0:128])
                # masked M0^T per s-tile
                m0m = T([128, 2, 256], "ssd_m0m")
                for st in range(2):
                    pm0 = P256()
                    nc.tensor.matmul(pm0[:], xcB[:, s, st * 128:(st + 1) * 128],
                                     xcC[:, s, :], start=True, stop=True)
                    V.tensor_mul(m0m[:, st, :], pm0[:], ct['maskT_spa'][:, st, :])
                # Acum transposes + strided copy
                acumT = T([128, 2, 4], "spa_acumT")
                for tt in range(2):
                    ptr = P256()
                    nc.tensor.transpose(ptr[:, 0:4],
                                        acum[:, s * 256 + tt * 128: s * 256 + (tt + 1) * 128],
                                        ident[0:4, 0:4])
                    S.copy(acumT[:, tt, :], ptr[:, 0:4])
                pb1 = P512()
                nc.tensor.matmul(pb1[:], ones4[0:1, :], aflat[:, s, 0:512],
                                 start=True, stop=True)
                pb2 = P512()
                nc.tensor.matmul(pb2[:], ones4[0:1, :], aflat[:, s, 512:1024],
                                 start=True, stop=True)
                # Y accumulation per head over s-tiles
                ypsl = [P256(), P256()]
                for st in range(2):
                    Dt = T([128, 4, 256], "ssd_Dt")
                    for h in range(H1):
                        pbx = pb1 if h < 2 else pb2
                        V.tensor_scalar(Dt[:, h, :],
                                        pbx[:, (h % 2) * 256:(h % 2 + 1) * 256],
                                        acumT[:, st, h:h + 1], 0.0,
                                        op0=ALU.subtract, op1=ALU.min)
                    Et = T([128, 4, 256], "ssd_Et")
                    S.activation(Et[:].rearrange("p h t -> p (h t)"),
                                 Dt[:].rearrange("p h t -> p (h t)"), AF.Exp)
                    MT = T([128, 4, 256], "ssd_MT")
                    V.tensor_tensor(MT[:], Et[:],
                                    m0m[:, st, :].unsqueeze(1).to_broadcast((128, 4, 256)),
                                    op=ALU.mult)
                    if i == 0 and s == 0 and st == 0:
                        tap("Dt00", lambda: ([128, 1024], lambda d: dma(
                            d[:], Dt[:].rearrange("p h t -> p (h t)"))))
                        tap("MT00", lambda: ([128, 1024], lambda d: dma(
                            d[:], MT[:].rearrange("p h t -> p (h t)"))))
                    for h in range(H1):
                        nc.tensor.matmul(ypsl[h // 2][(h % 2) * 64:(h % 2) * 64 + 64, :],
                                         xtm[:, st, h * 64:(h + 1) * 64],
                                         MT[:, h, :],
                                         start=(st == 0), stop=(st == 1),
                                         tile_position=(0, (h % 2) * 64),
                                         skip_group_check=True)
                if i == 0 and s == 0:
                    tap("xtm0", lambda: ([128, 512], lambda d: dma(
                        d[:], xtm[:].rearrange("p s t -> p (s t)"))))
                    tap("m0m0", lambda: ([128, 512], lambda d: dma(
                        d[:], m0m[:].rearrange("p s t -> p (s t)"))))
                    tap("acumT0", lambda: ([128, 8], lambda d: dma(
                        d[:], acumT[:].rearrange("p s t -> p (s t)"))))
                    tap("acs0", lambda: ([128, 256], lambda d: dma(d[:], acs[:])))
                ygt = T([128, 2, 256], "spa_ygt")
                y0t = T([128, 2, 256], "spa_y0t")
                for j in range(2):
                    V.scalar_tensor_tensor(y0t[:, j, :], xc[j][:, s, :],
                                           col(f"spa_dpc{i}_{j}"),
                                           ypsl[j][:], op0=ALU.mult, op1=ALU.add)
                    V.tensor_mul(ygt[:, j, :], y0t[:, j, :], zsil[:, j, s * 256:(s + 1) * 256])
                if i == 0 and s == 0:
                    tap("y00", lambda: ([128, 512], lambda d: dma(
                        d[:], y0t[:].rearrange("p j t -> p (j t)"))))
                    tap("zsil0", lambda: ([128, 1024], lambda d: dma(
                        d[:], zsil[:].rearrange("p j t -> p (j t)"))))
                # gated RMS norm over d_inner
                sqy = T([128, 2, 256], "sq_tmp")
                S.activation(sqy[:].rearrange("p j t -> p (j t)"),
                             ygt[:].rearrange("p j t -> p (j t)"), AF.Square)
                ssy = psS.tile([1, 256], F32, tag="small", name="small")
                for j in range(2):
                    nc.tensor.matmul(ssy[:], onescol, sqy[:, j, :],
                                     start=(j == 0), stop=(j == 1))
                varr = T([1, 256], "rowA")
                V.tensor_scalar(varr[:], ssy[:], 1.0 / 256, EPS, op0=ALU.mult, op1=ALU.add)
                rl = T([1, 256], "rowB")
                S.activation(rl[:], varr[:], AF.Ln)
                rrow = T([1, 256], "rowC")
                S.activation(rrow[:], rl[:], AF.Exp, scale=-0.5)
                rB = P256()
                nc.tensor.matmul(rB[:], onesrow1, rrow[:], start=True, stop=True)
                ynt = T([128, 2, 256], "spa_ynt")
                for j in range(2):
                    V.scalar_tensor_tensor(ynt[:, j, :], ygt[:, j, :],
                                           col(f"spa_rwc{i}_{j}"),
                                           rB[:], op0=ALU.mult, op1=ALU.mult)
                if i == 0 and s == 0:
                    tap("ygt0", lambda: ([128, 512], lambda d: dma(
                        d[:], ygt[:].rearrange("p s t -> p (s t)"))))
                    tap("ynt0", lambda: ([128, 512], lambda d: dma(
                        d[:], ynt[:].rearrange("p s t -> p (s t)"))))
                pop = P256()
                for j in range(2):
                    nc.tensor.matmul(pop[:], wt['spa_out_pk'][:, i, j, :], ynt[:, j, :],
                                     start=(j == 0), stop=(j == 1))
                V.tensor_add(h1[:, s, :], pop[:], xs[:, s, :])
            return h1

        # ================= spe mamba =================
        def spe_mamba(i, h1):
            # LayerNorm over the 256 features (free dim), batched samples
            mus = T([128, 2], "spe_mus")
            V.tensor_reduce(mus[:], h1[:], axis=AX.X, op=ALU.add)
            sq2 = T([128, 512], "sq_tmp")
            S.activation(sq2[:], h1[:].rearrange("p s t -> p (s t)"), AF.Square)
            ss2 = T([128, 2], "spe_ss2")
            V.tensor_reduce(ss2[:], sq2[:].rearrange("p (s t) -> p s t", s=2),
                            axis=AX.X, op=ALU.add)
            mean = T([128, 2], "spe_mean")
            V.tensor_scalar(mean[:], mus[:], 1.0 / 256, None, op0=ALU.mult)
            m2 = T([128, 2], "spe_m2")
            V.tensor_mul(m2[:], mean[:], mean[:])
            var2 = T([128, 2], "spe_var")
            V.scalar_tensor_tensor(var2[:], ss2[:], 1.0 / 256, m2[:],
                                   op0=ALU.mult, op1=ALU.subtract)
            l2t = T([128, 2], "spe_l2")
            S.activation(l2t[:], var2[:], AF.Ln, bias=epscol[:, 0:1])
            rstd2 = T([128, 2], "spe_rstd")
            S.activation(rstd2[:], l2t[:], AF.Exp, scale=-0.5)
            X2f = T([128, 2, 2, 128], "x2f_tmp")
            for s in range(BPC):
                xn = T([128, 256], "spe_xn")
                V.tensor_scalar(xn[:], h1[:, s, :], mean[:, s:s + 1], rstd2[:, s:s + 1],
                                op0=ALU.subtract, op1=ALU.mult)
                u = T([128, 256], "spe_u")
                V.tensor_mul(u[:], xn[:], wt['spe_ln_wB'][:, i, :])
                xsn = T([128, 256], "spe_xsn")
                V.tensor_add(xsn[:], u[:], wt['spe_ln_bB'][:, i, :])
                for ft in range(2):
                    ptr = P256()
                    nc.tensor.transpose(ptr[:, 0:128], xsn[:, ft * 128:(ft + 1) * 128],
                                        ident[:])
                    S.copy(X2f[:, s, ft, :], ptr[:, 0:128])
            # in_proj (samples batched along free): out cols ordered (s, t2)
            inw2t = T([128, 2, 1160], "w_spe_in")
            dma(inw2t[:], w_t['spe_in_pk'][i])
            inw2 = inw2t[:]
            ow2t = T([128, 4, 256], "w_spe_out")
            dma(ow2t[:], w_t['spe_out_pk'][i])
            ow2 = ow2t[:]

            def mm2(out_ap, off, width):
                for k in range(2):
                    nc.tensor.matmul(out_ap,
                                     inw2[:, k, off:off + width],
                                     X2f[:, :, k, :],
                                     start=(k == 0), stop=(k == 1))
            z2 = T([128, 4, 256], "mb_z")
            for j in range(4):
                pz = P256()
                mm2(pz[:], j * 128, 128)
                S.copy(z2[:, j, :], pz[:])
            cvx2 = []
            for j in range(4):
                px = P256()
                mm2(px[:], 512 + j * 128, 128)
                buf = T([128, 2, 131], f"cv_x{j}")
                V.memset(buf[:, :, 0:3], 0.0)
                S.copy(buf[:, :, 3:131], px[:].rearrange("p (s t) -> p s t", s=2))
                cvx2.append(buf)
            cvbc2 = []
            for nm, off in (("B", 1024), ("C", 1088)):
                pb = P256()
                for k in range(2):
                    nc.tensor.matmul(pb[0:64, :],
                                     inw2[:, k, off:off + 64],
                                     X2f[:, :, k, :], start=(k == 0), stop=(k == 1))
                buf = T([64, 2, 131], "cv_" + nm)
                V.memset(buf[:, :, 0:3], 0.0)
                S.copy(buf[:, :, 3:131], pb[0:64, :].rearrange("p (s t) -> p s t", s=2))
                cvbc2.append(buf)
            pdt = psS.tile([8, 256], F32, tag="small", name="small")
            for k in range(2):
                nc.tensor.matmul(pdt[:], inw2[:, k, 1152:1160],
                                 X2f[:, :, k, :], start=(k == 0), stop=(k == 1))
            e1 = T([8, 256], "rowA")
            S.activation(e1[:], pdt[:], AF.Exp, bias=col(f"spe_dtb{i}", 8))
            e1p = T([8, 256], "rowB")
            V.tensor_scalar(e1p[:], e1[:], 1.0, None, op0=ALU.add)
            dtv = T([8, 256], "mb_dtv")
            S.activation(dtv[:], e1p[:], AF.Ln)
            eA = T([8, 1], "spe_eA")
            S.activation(eA[:], col(f"spe_alog{i}", 8), AF.Exp)
            dtA = T([8, 256], "rowA")
            V.tensor_scalar(dtA[:], dtv[:], eA[:, 0:1], -1.0, op0=ALU.mult, op1=ALU.mult)
            acum = T([8, 256], "mb_acum")
            aflat = T([1, 2, 1024], "aflat")
            for s in range(BPC):
                V.tensor_tensor_scan(acum[:, s * 128:(s + 1) * 128],
                                     dtA[:, s * 128:(s + 1) * 128],
                                     dtA[:, s * 128:(s + 1) * 128], 0.0,
                                     op0=ALU.add, op1=ALU.bypass)
                dma(aflat[0:1, s, :].rearrange("o (p f) -> o p f", p=8),
                    acum[:, s * 128:(s + 1) * 128])
            # conv + silu
            xc2 = []
            for j in range(4):
                xc2.append(convchain(cvx2[j], wt['spe_conv_pk'][:, i, j, :],
                                     col(f"spe_cb{i}_{j}"), 128, 128, f"xc_{j}"))
            xcB = convchain(cvbc2[0], wt['spe_conv_pk'][0:64, i, 4, :],
                            col(f"spe_cbB{i}", 64), 64, 128, "xc_B")
            xcC = convchain(cvbc2[1], wt['spe_conv_pk'][0:64, i, 5, :],
                            col(f"spe_cbC{i}", 64), 64, 128, "xc_C")
            # z silu
            z2sil = T([128, 4, 256], "mb_zsil")
            for g in range(2):
                S.activation(z2sil[:, 2 * g:2 * g + 2, :].rearrange("p j t -> p (j t)"),
                             z2[:, 2 * g:2 * g + 2, :].rearrange("p j t -> p (j t)"),
                             AF.Silu)
            # dt-scaled x
            xp2 = T([128, 4, 256], "mb_xp")
            for j in range(4):
                pdb = P256()
                nc.tensor.matmul(pdb[:], ct['E_speJ'][:, j, :], dtv[:], start=True, stop=True)
                V.tensor_mul(xp2[:, j, :], xc2[j][:].rearrange("p s t -> p (s t)"), pdb[:])
            xs_new = T([128, 2, 256], "xs")
            for s in range(BPC):
                xtm2 = T([128, 512], "spe_xtm")
                for j in range(4):
                    ptr = P256()
                    nc.tensor.transpose(ptr[:, 0:128],
                                        xp2[:, j, s * 128:(s + 1) * 128], ident[:])
                    S.copy(xtm2[:, j * 128:(j + 1) * 128], ptr[:, 0:128])
                m0m2 = T([128, 128], "ssd_m0m")
                pm0 = P256()
                nc.tensor.matmul(pm0[:, 0:128], xcB[:, s, :], xcC[:, s, :],
                                 start=True, stop=True)
                V.tensor_mul(m0m2[:], pm0[:, 0:128], ct['maskT_spe'][:])
                acumT = T([128, 8], "spe_acumT")
                ptr = P256()
                nc.tensor.transpose(ptr[:, 0:8], acum[:, s * 128:(s + 1) * 128],
                                    ident[0:8, 0:8])
                S.copy(acumT[:], ptr[:, 0:8])
                pb1 = P512()
                nc.tensor.matmul(pb1[:], ones4[0:1, :], aflat[:, s, 0:512],
                                 start=True, stop=True)
                pb2 = P512()
                nc.tensor.matmul(pb2[:], ones4[0:1, :], aflat[:, s, 512:1024],
                                 start=True, stop=True)
                Dt = T([128, 8, 128], "ssd_Dt")
                for h in range(H2):
                    pbx = pb1 if h < 4 else pb2
                    V.tensor_scalar(Dt[:, h, :],
                                    pbx[:, (h % 4) * 128:(h % 4 + 1) * 128],
                                    acumT[:, h:h + 1], 0.0,
                                    op0=ALU.subtract, op1=ALU.min)
                Et = T([128, 8, 128], "ssd_Et")
                S.activation(Et[:].rearrange("p h t -> p (h t)"),
                             Dt[:].rearrange("p h t -> p (h t)"), AF.Exp)
                MT = T([128, 8, 128], "ssd_MT")
                V.tensor_tensor(MT[:], Et[:],
                                m0m2[:].unsqueeze(1).to_broadcast((128, 8, 128)),
                                op=ALU.mult)
                ygt2 = T([128, 4, 128], "spe_ygt")
                for j in range(4):
                    yp = P256()
                    for hh in range(2):
                        h = 2 * j + hh
                        nc.tensor.matmul(yp[hh * 64:hh * 64 + 64, 0:128],
                                         xtm2[:, h * 64:(h + 1) * 64],
                                         MT[:, h, :], start=True, stop=True,
                                         tile_position=(0, hh * 64),
                                         skip_group_check=True)
                    y0 = T([128, 128], "spe_y0")
                    V.scalar_tensor_tensor(y0[:], xc2[j][:, s, :],
                                           col(f"spe_dpc{i}_{j}"),
                                           yp[:, 0:128], op0=ALU.mult, op1=ALU.add)
                    V.tensor_mul(ygt2[:, j, :], y0[:], z2sil[:, j, s * 128:(s + 1) * 128])
                sqy = T([128, 4, 128], "sq_tmp")
                S.activation(sqy[:].rearrange("p j t -> p (j t)"),
                             ygt2[:].rearrange("p j t -> p (j t)"), AF.Square)
                ssy = psS.tile([1, 128], F32, tag="small", name="small")
                for j in range(4):
                    nc.tensor.matmul(ssy[:], onescol, sqy[:, j, :],
                                     start=(j == 0), stop=(j == 3))
                varr = T([1, 128], "rowA")
                V.tensor_scalar(varr[:], ssy[:], 1.0 / 512, EPS, op0=ALU.mult, op1=ALU.add)
                rl = T([1, 128], "rowB")
                S.activation(rl[:], varr[:], AF.Ln)
                rrow = T([1, 128], "rowC")
                S.activation(rrow[:], rl[:], AF.Exp, scale=-0.5)
                rB = P256()
                nc.tensor.matmul(rB[:, 0:128], onesrow1, rrow[:], start=True, stop=True)
                ynt = T([128, 4, 128], "spe_ynt")
                for j in range(4):
                    V.scalar_tensor_tensor(ynt[:, j, :], ygt2[:, j, :],
                                           col(f"spe_rwc{i}_{j}"),
                                           rB[:, 0:128], op0=ALU.mult, op1=ALU.mult)
                for ft in range(2):
                    ph2 = P256()
                    for k in range(4):
                        nc.tensor.matmul(ph2[:, 0:128],
                                         ow2[:, k, ft * 128:(ft + 1) * 128],
                                         ynt[:, k, :], start=(k == 0), stop=(k == 3))
                    h2sb = T([128, 128], "spe_h2sb")
                    S.copy(h2sb[:], ph2[:, 0:128])
                    ptr = P256()
                    nc.tensor.transpose(ptr[:, 0:128], h2sb[:], ident[:])
                    V.tensor_add(xs_new[:, s, ft * 128:(ft + 1) * 128], ptr[:, 0:128],
                                 h1[:, s, ft * 128:(ft + 1) * 128])
            return xs_new

        # ================= layers =================
        cur = xs
        for i in range(2):
            h1 = spa_mamba(i, cur)
            tap(f"h1_{i}", lambda: tap_batched(h1, [128, L]))
            cur = spe_mamba(i, h1)
            tap(f"xsl{i + 1}", lambda: tap_batched(cur, [128, L]))

        # ================= final LN =================
        xfl = part_ln(cur[:].rearrange("p s t -> p (s t)"), 2)
        xf = xfl[:].rearrange("p (s t) -> p s t", s=BPC)
        tap("xf", lambda: ([BPC, 128, L],
                           lambda d: [dma(d[s], xf[:, s, :]) for s in range(BPC)]))

        # ================= spa attention (center query) =================
        pctr = psS.tile([128, 2], F32, tag="small", name="small")
        for l in range(5):
            nc.tensor.matmul(pctr[:], wt['cprj_pk'][:, l, :], xf[:, :, l],
                             start=(l == 0), stop=(l == 4))
        ctr = T([128, 2], "at_ctr")
        S.activation(ctr[:], pctr[:], AF.Identity, bias=col("cprj_b"))
        pq = psS.tile([128, 2], F32, tag="small", name="small")
        nc.tensor.matmul(pq[:], wt['aqT'][:], ctr[:], start=True, stop=True)
        qsb = T([128, 2], "at_q")
        S.activation(qsb[:], pq[:], AF.Identity, bias=col("aq_b"))
        pk = P512()
        nc.tensor.matmul(pk[:], wt['akT'][:], xfl[:], start=True, stop=True)
        Ksb = T([128, 2, 256], "at_K")
        S.activation(Ksb[:].rearrange("p s t -> p (s t)"), pk[:], AF.Identity,
                     bias=col("ak_b"))
        pv = P512()
        nc.tensor.matmul(pv[:], wt['avT'][:], xfl[:], start=True, stop=True)
        Vsb = T([128, 2, 256], "at_V")
        S.activation(Vsb[:].rearrange("p s t -> p (s t)"), pv[:], AF.Identity,
                     bias=col("av_b"))
        vo = T([128, 2, 256], "at_vo")
        for s in range(BPC):
            qd = T([128, 8], "at_qd")
            V.tensor_tensor(qd[:], qsb[:, s:s + 1].to_broadcast((128, 8)),
                            ct['Emask_q'][:], op=ALU.mult)
            plg = psS.tile([8, 256], F32, tag="small", name="small")
            nc.tensor.matmul(plg[:], qd[:], Ksb[:, s, :], start=True, stop=True)
            nm = T([8, 1], "at_nm")
            V.tensor_reduce(nm[:], plg[:], axis=AX.X, op=ALU.max, negate=True)
            nm4 = T([8, 1], "at_nm4")
            V.tensor_scalar(nm4[:], nm[:], 0.25, None, op0=ALU.mult)
            ex = T([8, 256], "at_ex")
            S.activation(ex[:], plg[:], AF.Exp, bias=nm4[:, 0:1], scale=0.25)
            sm = T([8, 1], "at_sm")
            V.tensor_reduce(sm[:], ex[:], axis=AX.X, op=ALU.add)
            rc = T([8, 1], "at_rc")
            V.reciprocal(rc[:], sm[:])
            aw = T([8, 256], "at_aw")
            V.tensor_scalar(aw[:], ex[:], rc[:, 0:1], None, op0=ALU.mult)
            patB = P256()
            nc.tensor.matmul(patB[:], ct['E_attn'][:], aw[:], start=True, stop=True)
            V.tensor_mul(vo[:, s, :], Vsb[:, s, :], patB[:])
        pao = P512()
        nc.tensor.matmul(pao[:], wt['aoT'][:], vo[:].rearrange("p s t -> p (s t)"),
                         start=True, stop=True)
        xa = T([128, 2, 256], "xa")
        V.scalar_tensor_tensor(xa[:].rearrange("p s t -> p (s t)"), pao[:],
                               col("ao_b"), xfl[:], op0=ALU.add, op1=ALU.add)
        tap("xa", lambda: tap_batched(xa, [128, L]))

        # ================= spe attention =================
        X2a = T([128, 2, 2, 128], "x2f_tmp")
        for s in range(BPC):
            for ft in range(2):
                ptr = P256()
                nc.tensor.transpose(ptr[:, 0:128], xa[:, s, ft * 128:(ft + 1) * 128],
                                    ident[:])
                S.copy(X2a[:, s, ft, :], ptr[:, 0:128])
        q2 = T([128, 2, 2, 128], "sp2_q2")
        k2 = T([128, 2, 2, 128], "sp2_k2")
        for s in range(BPC):
            for ot in range(2):
                pq2 = P256()
                for ft in range(2):
                    nc.tensor.matmul(pq2[:, 0:128],
                                     wt['sqT'][:, ft, ot * 128:(ot + 1) * 128],
                                     X2a[:, s, ft, :], start=(ft == 0), stop=(ft == 1))
                S.activation(q2[:, s, ot, :], pq2[:, 0:128], AF.Identity,
                             bias=col(f"sq_b{ot}"))
                pk2 = P256()
                for ft in range(2):
                    nc.tensor.matmul(pk2[:, 0:128],
                                     wt['skT'][:, ft, ot * 128:(ot + 1) * 128],
                                     X2a[:, s, ft, :], start=(ft == 0), stop=(ft == 1))
                S.activation(k2[:, s, ot, :], pk2[:, 0:128], AF.Identity,
                             bias=col(f"sk_b{ot}"))
        xs2 = T([128, 2, 256], "xs2")
        for s in range(BPC):
            pv2 = P256()
            for ft in range(2):
                nc.tensor.matmul(pv2[:], X2a[:, s, ft, :], wt['svT'][:, ft, :],
                                 start=(ft == 0), stop=(ft == 1))
            v2 = T([128, 256], "sp2_v2")
            V.tensor_add(v2[:], pv2[:], wt['svbB'][:])
            pa2 = P256()
            for ot in range(2):
                nc.tensor.matmul(pa2[:, 0:128], q2[:, s, ot, :], k2[:, s, ot, :],
                                 start=(ot == 0), stop=(ot == 1))
            nm = T([128, 1], "sp2_nm")
            V.tensor_reduce(nm[:], pa2[:, 0:128], axis=AX.X, op=ALU.max, negate=True)
            nm16 = T([128, 1], "sp2_nm16")
            V.tensor_scalar(nm16[:], nm[:], 1.0 / 16, None, op0=ALU.mult)
            ex = T([128, 128], "sp2_ex")
            S.activation(ex[:], pa2[:, 0:128], AF.Exp, bias=nm16[:, 0:1], scale=1.0 / 16)
            sm = T([128, 1], "sp2_sm")
            V.tensor_reduce(sm[:], ex[:], axis=AX.X, op=ALU.add)
            rc = T([128, 1], "sp2_rc")
            V.reciprocal(rc[:], sm[:])
            a2 = T([128, 128], "sp2_a2")
            V.tensor_scalar(a2[:], ex[:], rc[:, 0:1], None, op0=ALU.mult)
            pa2T = P256()
            nc.tensor.transpose(pa2T[:, 0:128], a2[:], ident[:])
            a2T = T([128, 128], "sp2_a2T")
            S.copy(a2T[:], pa2T[:, 0:128])
            o2 = T([128, 2, 128], "sp2_o2")
            for ot in range(2):
                po2 = P256()
                nc.tensor.matmul(po2[:, 0:128], v2[:, ot * 128:(ot + 1) * 128], a2T[:],
                                 start=True, stop=True)
                S.copy(o2[:, ot, :], po2[:, 0:128])
            po3 = P256()
            for ot in range(2):
                nc.tensor.matmul(po3[:], o2[:, ot, :], wt['soT'][:, ot, :],
                                 start=(ot == 0), stop=(ot == 1))
            t3 = T([128, 256], "sp2_t3")
            V.tensor_add(t3[:], po3[:], wt['sobB'][:])
            V.tensor_add(xs2[:, s, :], t3[:], xa[:, s, :])
        tap("xs2", lambda: tap_batched(xs2, [128, L]))

        # ================= downsample =================
        pds = psD.tile([64, 256], F32, tag="ds", name="ds")
        invr = T([1, BPC, L], "irow_raw", I32)
        dma(invr[:], inv[None, :, :])
        invf = T([1, BPC, L], "irow_f")
        V.tensor_copy(invf[:], invr[:])
        for s in range(BPC):
            # inverse permutation (argsort-based) one-hot
            invB = P512()
            nc.tensor.matmul(invB[:, 0:L], onesrow1, invf[:, s, :], start=True, stop=True)
            QT = T([128, 2, 256], "perm_oh")
            for tt in range(2):
                V.tensor_scalar(QT[:, tt, :], invB[:, 0:L], ct['iotaC'][:, tt:tt + 1],
                                None, op0=ALU.is_equal)
            tmv = T([128, 2, 128], "tm_tmp")
            for tt in range(2):
                ptr = P256()
                nc.tensor.transpose(ptr[:, 0:128], xs2[:, s, tt * 128:(tt + 1) * 128],
                                    ident[:])
                S.copy(tmv[:, tt, :], ptr[:, 0:128])
            pxr = P256()
            for tt in range(2):
                nc.tensor.matmul(pxr[:], tmv[:, tt, :], QT[:, tt, :],
                                 start=(tt == 0), stop=(tt == 1))
            xrp = T([128, 324], "ds_xrp")
            V.memset(xrp[:], 0.0)
            xr3 = xrp[:].rearrange("p (h w) -> p h w", h=18)
            S.copy(xr3[:, 1:17, 1:17], pxr[:].rearrange("p (h w) -> p h w", h=16))
            for kh in range(3):
                for kw in range(3):
                    k = kh * 3 + kw
                    cmp_ = T([128, 64], "ds_cmp")
                    V.tensor_copy(cmp_[:].rearrange("p (a b) -> p a b", a=8),
                                  xr3[:, kh:kh + 16:2, kw:kw + 16:2])
                    nc.tensor.matmul(pds[:, s * 128:(s + 1) * 128],
                                     cmp_[:],
                                     wt['dsw_pk'][:, k, :],
                                     start=(k == 0), stop=(k == 8),
                                     skip_group_check=True)
        for s in range(BPC):
            view = pds[:, s * 128:(s + 1) * 128]
            mus = T([64, 1], "ds_mus")
            V.tensor_reduce(mus[:], view, axis=AX.X, op=ALU.add)
            mean = T([64, 1], "ds_mean")
            V.tensor_scalar(mean[:], mus[:], 1.0 / 128, None, op0=ALU.mult)
            sq = T([64, 128], "sq_tmp")
            S.activation(sq[:], view, AF.Square)
            ss = T([64, 1], "ds_ss")
            V.tensor_reduce(ss[:], sq[:], axis=AX.X, op=ALU.add)
            m2 = T([64, 1], "ds_m2")
            V.tensor_mul(m2[:], mean[:], mean[:])
            var = T([64, 1], "ds_var")
            V.scalar_tensor_tensor(var[:], ss[:], 1.0 / 128, m2[:],
                                   op0=ALU.mult, op1=ALU.subtract)
            lv = T([64, 1], "ds_lv")
            S.activation(lv[:], var[:], AF.Ln, bias=epscol[0:64, 0:1])
            rstd = T([64, 1], "ds_rstd")
            S.activation(rstd[:], lv[:], AF.Exp, scale=-0.5)
            xn = T([64, 128], "ds_xn")
            V.tensor_scalar(xn[:], view, mean[:, 0:1], rstd[:, 0:1],
                            op0=ALU.subtract, op1=ALU.mult)
            t1 = T([64, 128], "ds_t1")
            V.tensor_mul(t1[:], xn[:], wt['ds_ln_wB'][:])
            o1 = T([64, 128], "ds_o1")
            V.tensor_add(o1[:], t1[:], wt['ds_ln_bB'][:])
            dma(out[s].rearrange("h w c -> (h w) c"), o1[:])

        stk.close()
    return nc, tap_t


# ---------------------------------------------------------------------------
_CACHE = {}


def _get_program(taps=()):
    key = tuple(sorted(taps))
    if key not in _CACHE:
        _CACHE[key] = build_program(taps)
    return _CACHE[key]


def make_inmaps(inputs, taps=()):
    cst = host_constants()
    w = prep_weights(inputs)
    x = np.asarray(inputs['x'], np.float32).reshape(16, C, L)
    idx = np.asarray(inputs['sorted_index'], np.int32)
    inv = np.argsort(idx, axis=1, kind='stable').astype(np.int32)
    in_maps = []
    for c in range(NCORES):
        m = {}
        m.update({k: np.ascontiguousarray(v, np.float32) for k, v in cst.items()})
        m.update({k: np.ascontiguousarray(v, np.float32) for k, v in w.items()})
        sl = slice(c * BPC, (c + 1) * BPC)
        m['x2'] = np.ascontiguousarray(x[sl])
        m['idx'] = np.ascontiguousarray(idx[sl])
        m['inv'] = np.ascontiguousarray(inv[sl])
        in_maps.append(m)
    return in_maps


def run(inputs, taps=(), trace=False):
    nc, tap_t = _get_program(taps)
    in_maps = make_inmaps(inputs, taps)
    res = run_bass_kernel_spmd(nc, in_maps, list(range(NCORES)), trace=trace)
    outs = np.concatenate([r['out'] for r in res.results], axis=0)
    tapd = {}
    for name in taps:
        tapd[name] = [r.get('t_' + name) for r in res.results]
    return outs, tapd, res


def kernel(**inputs):
    outs, _, _ = run(inputs)
    return outs



# revision 38
# speedup vs baseline: 1.3259x; 1.3259x over previous
"""Trainium2 Bass kernel for nn_Basic_Block_v1 (spatial/spectral Mamba2 block).

Sharding: data-parallel over batch (16 samples) across 8 NeuronCores,
2 samples per core; all parameters replicated. Heavy math in bf16 on the
TensorEngine (1 cyc/row vs 4 for fp32); the SSD decay cumsum path stays fp32.
Depthwise convs are folded into the in_proj matmuls via host-side weight
scaling with shifted moving operands (zero-padded token axis).
"""
import sys
sys.path.insert(0, '/opt/trn_rl_repo')
import json

import numpy as np
import ml_dtypes

import concourse.bass as bass
import concourse.mybir as mybir
from concourse import tile
from concourse.bass_utils import run_bass_kernel_spmd

F32 = mybir.dt.float32
BF16 = mybir.dt.bfloat16
I32 = mybir.dt.int32
AF = mybir.ActivationFunctionType
ALU = mybir.AluOpType
AX = mybir.AxisListType
NPBF = ml_dtypes.bfloat16

NCORES = 8
BPC = 2          # batch per core
L = 256          # spatial tokens
C = 128          # channels
H1 = 4           # spa heads
H2 = 8           # spe heads
EPS = 1e-5
NEG = -88.0

# ---------------------------------------------------------------------------
# walrus in this container supports only ONE sync-wait per instruction;
# split extra waits emitted by the Tile scheduler onto preceding NoOps.
_WAIT_LIMIT = 1
_orig_to_json = bass.Bass.to_json_bytes


def _fix_block(b, ctr):
    insts = b.get('instructions')
    if insts:
        out = []
        for ins in insts:
            si = ins.get('sync_info')
            waits = (si or {}).get('on_wait') or []
            if len(waits) > _WAIT_LIMIT:
                while len(waits) > _WAIT_LIMIT:
                    chunk, waits = waits[:_WAIT_LIMIT], waits[_WAIT_LIMIT:]
                    ctr[0] += 1
                    out.append({
                        "debug": ins.get("debug"),
                        "engine": ins["engine"],
                        "ins": [],
                        "name": f"I-wsplit{ctr[0]}",
                        "opcode": "NoOp",
                        "outs": [],
                        "text_hint": "wsplit",
                        "sync_info": {"on_update": [], "on_wait": chunk},
                    })
                si['on_wait'] = waits
            out.append(ins)
        b['instructions'] = out
    for sb in b.get('blocks') or []:
        _fix_block(sb, ctr)


def _patched_to_json(self, *a, **k):
    raw = _orig_to_json(self, *a, **k)
    d = json.loads(raw)
    ctr = [0]
    for f in d.get('functions', []):
        for b in f.get('blocks', []):
            _fix_block(b, ctr)
    if ctr[0] == 0:
        return raw
    return json.dumps(d).encode()


bass.Bass.to_json_bytes = _patched_to_json


# ---------------------------------------------------------------------------
def _sincos_2d(dim, Hg):
    def e1(d, pos):
        omega = 1.0 / (10000.0 ** (np.arange(d // 2, dtype=np.float64) / (d / 2.0)))
        out = pos[:, None] * omega[None, :]
        return np.concatenate([np.sin(out), np.cos(out)], axis=-1)
    gh, gw = np.meshgrid(np.arange(Hg), np.arange(Hg), indexing='ij')
    emb = np.concatenate([e1(dim // 2, gh.reshape(-1)), e1(dim // 2, gw.reshape(-1))], axis=-1)
    return emb.astype(np.float32)


def host_constants():
    d = {}
    d['pe_fm'] = np.ascontiguousarray(_sincos_2d(C, 16).T).astype(NPBF)   # [128, 256]
    d['ident'] = np.eye(128, dtype=np.float32).astype(NPBF)
    d['identF'] = np.eye(16, dtype=np.float32)
    iota = np.arange(L, dtype=np.float32)
    d['iotaC'] = np.stack([iota[:128], iota[128:]], axis=1).copy()        # [128, 2] f32
    # Minf[sp][st][t] = 0 if (st*128+sp) <= t else NEG   (spa, L=256)
    sidx = np.arange(L)[:, None]
    tidx = np.arange(L)[None, :]
    m = np.where(sidx <= tidx, 0.0, NEG).astype(np.float32)               # [s, t]
    d['minf_spa'] = np.stack([m[:128], m[128:]], axis=1).astype(NPBF)     # [128, 2, 256]
    s2 = np.arange(C)[:, None]
    t2 = np.arange(C)[None, :]
    d['minf_spe'] = np.where(s2 <= t2, 0.0, NEG).astype(NPBF)             # [128, 128]
    EA = np.zeros((8, 128), np.float32)
    for h in range(8):
        EA[h, h * 16:(h + 1) * 16] = 1.0
    d['E_attn'] = EA.astype(NPBF)                                         # [8, 128]
    d['Emask_q'] = EA.T.copy().astype(NPBF)                               # [128, 8]
    return d


COL_ORDER = (
    [f"spa_dtb{i}" for i in range(2)] + [f"spa_alog{i}" for i in range(2)]
    + [f"spa_cb{i}_{b}" for i in range(2) for b in range(2)]
    + [f"spa_cbB{i}" for i in range(2)] + [f"spa_cbC{i}" for i in range(2)]
    + [f"spa_dpc{i}_{j}" for i in range(2) for j in range(2)]
    + [f"spa_rwc{i}_{j}" for i in range(2) for j in range(2)]
    + [f"spe_dtb{i}" for i in range(2)] + [f"spe_alog{i}" for i in range(2)]
    + [f"spe_cb{i}_{b}" for i in range(2) for b in range(4)]
    + [f"spe_cbB{i}" for i in range(2)] + [f"spe_cbC{i}" for i in range(2)]
    + [f"spe_dpc{i}_{j}" for i in range(2) for j in range(4)]
    + [f"spe_rwc{i}_{j}" for i in range(2) for j in range(4)]
    + ["lnw_spa0", "lnw_spa1", "lnw_norm",
       "cprj_b", "aq_b", "ak_b", "av_b", "ao_b",
       "sq_b0", "sq_b1", "sk_b0", "sk_b1"]
)
CIDX = {k: ix for ix, k in enumerate(COL_ORDER)}


def prep_weights(inp):
    """Host-side layout prep: bf16 casts, transposes, conv folding."""
    w = {}
    bf = lambda a: np.ascontiguousarray(a).astype(NPBF)
    # ---- spa mamba: in_w [2, 644, 128]; conv folded into xBC blocks ----
    spa_z = np.zeros((2, 128, 256), np.float32)
    spa_xc = np.zeros((2, 4, 128, 384), np.float32)
    spa_dt = np.zeros((2, 128, 4), np.float32)
    for i in range(2):
        W = np.asarray(inp['spa_in_w'][i], np.float32)          # [644, 128]
        cw = np.asarray(inp['spa_conv_w'][i], np.float32)       # [384, 4]
        spa_z[i] = W[0:256].T
        for k in range(4):
            spa_xc[i, k] = (W[256:640] * cw[:, k:k + 1]).T
        spa_dt[i] = W[640:644].T
    w['spa_z_w'] = bf(spa_z.transpose(1, 0, 2))          # [128, 2, 256]
    w['spa_xc_w'] = bf(spa_xc.transpose(2, 0, 1, 3))     # [128, 2, 4, 384]
    w['spa_dt_w'] = bf(spa_dt.transpose(1, 0, 2))        # [128, 2, 4]
    sow = np.transpose(inp['spa_out_w'], (0, 2, 1)).reshape(2, 2, 128, 128)
    w['spa_out_pk'] = bf(np.transpose(sow, (2, 0, 1, 3)))       # [128, 2, 2, 128]
    # ---- spe mamba: in_w [2, 1160, 256] ----
    spe_z = np.zeros((2, 2, 128, 512), np.float32)
    spe_xc = np.zeros((2, 4, 2, 128, 640), np.float32)
    spe_dt = np.zeros((2, 2, 128, 8), np.float32)
    for i in range(2):
        W = np.asarray(inp['spe_in_w'][i], np.float32)          # [1160, 256]
        cw = np.asarray(inp['spe_conv_w'][i], np.float32)       # [640, 4]
        for kc in range(2):
            cs = slice(kc * 128, (kc + 1) * 128)
            spe_z[i, kc] = W[0:512, cs].T
            spe_dt[i, kc] = W[1152:1160, cs].T
            for k in range(4):
                spe_xc[i, k, kc] = (W[512:1152, cs] * cw[:, k:k + 1]).T
    w['spe_z_w'] = bf(spe_z.transpose(2, 0, 1, 3))       # [128, 2, 2, 512]
    w['spe_xc_w'] = bf(spe_xc.transpose(3, 0, 1, 2, 4))  # [128, 2, 4, 2, 640]
    w['spe_dt_w'] = bf(spe_dt.transpose(2, 0, 1, 3))     # [128, 2, 2, 8]
    sew = np.transpose(inp['spe_out_w'], (0, 2, 1)).reshape(2, 4, 128, 256)
    w['spe_out_pk'] = bf(np.transpose(sew, (2, 0, 1, 3)))       # [128, 2, 4, 256]
    w['spe_ln_wB'] = bf(np.broadcast_to(
        inp['spe_ln_w'][:, None, :], (2, 128, 256)).transpose(1, 0, 2))
    w['spe_ln_bB'] = bf(np.broadcast_to(
        inp['spe_ln_b'][:, None, :], (2, 128, 256)).transpose(1, 0, 2))
    # ---- attention / head ----
    w['cprj_pk'] = bf(np.transpose(inp['cprj_w'], (2, 1, 0)).transpose(1, 0, 2))
    for nm in ('aq', 'ak', 'av', 'ao'):
        w[nm + 'T'] = bf(inp[nm + '_w'].T)
    for nm in ('sq', 'sk', 'sv', 'so'):
        wt_ = np.asarray(inp[nm + '_w'], np.float32).T.reshape(2, 128, 256)
        w[nm + 'T'] = bf(wt_.transpose(1, 0, 2))
    w['svbB'] = bf(np.broadcast_to(inp['sv_b'][None, :], (128, 256)))
    w['sobB'] = bf(np.broadcast_to(inp['so_b'][None, :], (128, 256)))
    w['dsw_pk'] = bf(np.asarray(inp['ds_conv_w'], np.float32)
                     .reshape(9, 128, 128).transpose(1, 0, 2))
    w['ds_ln_wB'] = np.ascontiguousarray(
        np.broadcast_to(inp['ds_ln_w'][None, :], (64, 128))).astype(np.float32)
    w['ds_ln_bB'] = np.ascontiguousarray(
        np.broadcast_to(inp['ds_ln_b'][None, :], (64, 128))).astype(np.float32)
    # partition-dim layernorm stationary: row0 = -w (sign trick), row1 = b
    lnwb = np.zeros((2, 3, 128), np.float32)
    lnwb[0, 0], lnwb[1, 0] = -np.asarray(inp['spa_ln_w'][0]), inp['spa_ln_b'][0]
    lnwb[0, 1], lnwb[1, 1] = -np.asarray(inp['spa_ln_w'][1]), inp['spa_ln_b'][1]
    lnwb[0, 2], lnwb[1, 2] = -np.asarray(inp['norm_w']), inp['norm_b']
    w['lnwb'] = bf(lnwb)
    # ---- f32 scalar column pack ----
    cols = {}
    for i in range(2):
        cols[f"spa_dtb{i}"] = inp['spa_dt_bias'][i]
        cols[f"spa_alog{i}"] = inp['spa_A_log'][i]
        cb = np.asarray(inp['spa_conv_b'][i], np.float32)
        cols[f"spa_cb{i}_0"] = cb[0:128]
        cols[f"spa_cb{i}_1"] = cb[128:256]
        cols[f"spa_cbB{i}"] = cb[256:320]
        cols[f"spa_cbC{i}"] = cb[320:384]
        for j in range(2):
            cols[f"spa_dpc{i}_{j}"] = np.repeat(inp['spa_D'][i], 64)[j * 128:(j + 1) * 128]
            cols[f"spa_rwc{i}_{j}"] = inp['spa_rms_w'][i, j * 128:(j + 1) * 128]
        cols[f"spe_dtb{i}"] = inp['spe_dt_bias'][i]
        cols[f"spe_alog{i}"] = inp['spe_A_log'][i]
        cb2 = np.asarray(inp['spe_conv_b'][i], np.float32)
        for b in range(4):
            cols[f"spe_cb{i}_{b}"] = cb2[b * 128:(b + 1) * 128]
        cols[f"spe_cbB{i}"] = cb2[512:576]
        cols[f"spe_cbC{i}"] = cb2[576:640]
        for j in range(4):
            cols[f"spe_dpc{i}_{j}"] = np.repeat(inp['spe_D'][i], 64)[j * 128:(j + 1) * 128]
            cols[f"spe_rwc{i}_{j}"] = inp['spe_rms_w'][i, j * 128:(j + 1) * 128]
    cols["lnw_spa0"] = inp['spa_ln_w'][0]
    cols["lnw_spa1"] = inp['spa_ln_w'][1]
    cols["lnw_norm"] = inp['norm_w']
    cols["cprj_b"] = inp['cprj_b']
    for nm in ('aq', 'ak', 'av', 'ao'):
        cols[nm + "_b"] = inp[nm + '_b']
    cols["sq_b0"] = inp['sq_b'][0:128]
    cols["sq_b1"] = inp['sq_b'][128:256]
    cols["sk_b0"] = inp['sk_b'][0:128]
    cols["sk_b1"] = inp['sk_b'][128:256]
    pk = np.zeros((128, len(COL_ORDER)), np.float32)
    for k, v in cols.items():
        v = np.asarray(v, np.float32)
        pk[0:v.shape[0], CIDX[k]] = v
    w['colpak'] = pk
    return w


WSHAPES = {
    'spa_z_w': ([128, 2, 256], BF16), 'spa_xc_w': ([128, 2, 4, 384], BF16),
    'spa_dt_w': ([128, 2, 4], BF16), 'spa_out_pk': ([128, 2, 2, 128], BF16),
    'spe_z_w': ([128, 2, 2, 512], BF16), 'spe_xc_w': ([128, 2, 4, 2, 640], BF16),
    'spe_dt_w': ([128, 2, 2, 8], BF16), 'spe_out_pk': ([128, 2, 4, 256], BF16),
    'spe_ln_wB': ([128, 2, 256], BF16), 'spe_ln_bB': ([128, 2, 256], BF16),
    'cprj_pk': ([128, 5, 128], BF16),
    'aqT': ([128, 128], BF16), 'akT': ([128, 128], BF16),
    'avT': ([128, 128], BF16), 'aoT': ([128, 128], BF16),
    'sqT': ([128, 2, 256], BF16), 'skT': ([128, 2, 256], BF16),
    'svT': ([128, 2, 256], BF16), 'soT': ([128, 2, 256], BF16),
    'svbB': ([128, 256], BF16), 'sobB': ([128, 256], BF16),
    'dsw_pk': ([128, 9, 128], BF16),
    'ds_ln_wB': ([64, 128], F32), 'ds_ln_bB': ([64, 128], F32),
    'lnwb': ([2, 3, 128], BF16), 'colpak': ([128, len(COL_ORDER)], F32),
}
CSHAPES = {
    'pe_fm': ([128, 256], BF16), 'ident': ([128, 128], BF16),
    'identF': ([16, 16], F32), 'iotaC': ([128, 2], F32),
    'minf_spa': ([128, 2, 256], BF16), 'minf_spe': ([128, 128], BF16),
    'E_attn': ([8, 128], BF16), 'Emask_q': ([128, 8], BF16),
}

# ---------------------------------------------------------------------------
def build_program(taps=()):
    """Builds the per-core SPMD Bass program."""
    nc = bass.Bass()

    def din(name, shape, dt=F32):
        return nc.dram_tensor(name, shape, dt, kind="ExternalInput")

    x2 = din("x2", [BPC, C, L], BF16)
    idx = din("idx", [BPC, L], I32)
    inv = din("inv", [BPC, L], I32)
    cst_t = {k: din(k, shp, dt) for k, (shp, dt) in CSHAPES.items()}
    w_t = {k: din(k, shp, dt) for k, (shp, dt) in WSHAPES.items()}
    out = nc.dram_tensor("out", [BPC, 8, 8, C], F32, kind="ExternalOutput")
    tap_t = {}

    with tile.TileContext(nc) as tc:
        import contextlib
        stk = contextlib.ExitStack()
        sb = stk.enter_context(tc.tile_pool(name="sb", bufs=2))
        wb = stk.enter_context(tc.tile_pool(name="wb", bufs=1))
        psA = stk.enter_context(tc.tile_pool(name="psA", bufs=2, space="PSUM"))
        psB = stk.enter_context(tc.tile_pool(name="psB", bufs=3, space="PSUM"))
        psS = stk.enter_context(tc.tile_pool(name="psS", bufs=2, space="PSUM"))
        psD = stk.enter_context(tc.tile_pool(name="psD", bufs=1, space="PSUM"))

        def T(shape, tag, dt=BF16, bufs=None):
            return sb.tile(shape, dt, tag=tag, name=tag, bufs=bufs)

        def W(shape, tag, dt=BF16):
            return wb.tile(shape, dt, tag=tag, name=tag, bufs=1)

        def PA(shape=(128, 512), dt=F32):
            return psA.tile(list(shape), dt, tag="A", name="pa",
                            padded_shape=[128, 512 if dt == F32 else 1024])

        def PB(shape=(128, 512), dt=F32):
            return psB.tile(list(shape), dt, tag="B", name="pb",
                            padded_shape=[128, 512 if dt == F32 else 1024])

        def PS(shape=(16, 512), dt=F32):
            return psS.tile(list(shape), dt, tag="S", name="ps",
                            padded_shape=[shape[0], 512])

        dma = nc.sync.dma_start
        V = nc.vector
        S = nc.scalar
        MM = nc.tensor.matmul
        TR = nc.tensor.transpose

        # ---------- load constants + weights (single DMA per tensor) ----------
        ct = {}
        for k, (shp, dt) in CSHAPES.items():
            ct[k] = W(shp, "c_" + k, dt)
            dma(ct[k][:], cst_t[k][:])
        wt = {}
        for k, (shp, dt) in WSHAPES.items():
            wt[k] = W(shp, "w_" + k, dt)
            dma(wt[k][:], w_t[k][:])
        colpak = wt['colpak']

        def col(key, p=128):
            return colpak[0:p, CIDX[key]:CIDX[key] + 1]

        onesB = W([128, 128], "onesB", BF16)
        V.memset(onesB[:], 1.0)
        onescolB = onesB[:, 0:1]
        onesrowB = onesB[0:1, :]
        onesF = W([1, 128], "onesF", F32)
        V.memset(onesF[:], 1.0)
        onecolF = W([128, 1], "onecolF", F32)
        V.memset(onecolF[:], 1.0)
        epscol = W([128, 1], "epscol", F32)
        V.memset(epscol[:], EPS)
        ident = ct['ident']
        identF = ct['identF']

        def tap(name, src_ap, shape):
            # stage through f32 + DMA out (debug only)
            if name not in taps:
                return
            st_ = T(list(shape), "tapstage", F32)
            S.copy(st_[:], src_ap)
            t = nc.dram_tensor("t_" + name, list(shape), F32, kind="ExternalOutput")
            tap_t[name] = t
            dma(t[:], st_[:])

        # ---------- stage 0: embed + permute ----------
        xb = T([128, BPC, L], "xb")
        for s in range(BPC):
            dma(xb[:, s, :], x2[s])
        x0 = T([128, BPC, L], "x0")
        V.tensor_tensor(
            x0[:], xb[:],
            ct['pe_fm'][:].unsqueeze(1).to_broadcast((128, BPC, L)),
            op=ALU.add)

        idxr = T([1, BPC, L], "irow_raw", I32, bufs=1)
        dma(idxr[:], idx[None, :, :])
        idxf = T([1, BPC, L], "irow_f", F32, bufs=1)
        V.tensor_copy(idxf[:], idxr[:])

        xs = T([128, BPC, L], "xs")
        for s in range(BPC):
            idxB = PB()
            MM(idxB[:, 0:L], onesF[:], idxf[:, s, :], start=True, stop=True)
            PmT = T([128, 2, L], "perm_oh")
            for st in range(2):
                V.tensor_scalar(PmT[:, st, :], idxB[:, 0:L], ct['iotaC'][:, st:st + 1],
                                None, op0=ALU.is_equal)
            x0tm = T([128, 2, 128], "tm_tmp")
            for tt in range(2):
                ptr = PB((128, 128), BF16)
                TR(ptr[:, 0:128], x0[:, s, tt * 128:(tt + 1) * 128], ident[:])
                S.copy(x0tm[:, tt, :], ptr[:, 0:128])
            pxs = PB((128, 256))
            for st in range(2):
                MM(pxs[:], x0tm[:, st, :], PmT[:, st, :],
                   start=(st == 0), stop=(st == 1))
            S.copy(xs[:, s, :], pxs[:])
        tap("xs0", xs[:].rearrange("p s t -> p (s t)"), (128, 512))

        # ================= partition-dim layernorm =================
        lnrhs = T([2, 512], "ln_rhs", BF16, bufs=2)

        def part_ln(xsrc, lnidx, dst):
            """LN over channel (partition) dim. xsrc/dst: [128, 2, 256] views."""
            xflat = xsrc.rearrange("p s t -> p (s t)") if len(xsrc.shape) == 3 else xsrc
            sq = T([128, 512], "ln_sq")
            S.activation(sq[:], xflat, AF.Square)
            msum = PS((1, 512))
            MM(msum[:], onescolB, xflat, start=True, stop=True)
            ssum = PS((1, 512))
            MM(ssum[:], onescolB, sq[:], start=True, stop=True)
            murow = T([1, 512], "ln_mu", F32, bufs=1)
            S.activation(murow[:], msum[:], AF.Copy, scale=1.0 / 128)
            mu2 = T([1, 512], "ln_mu2", F32, bufs=1)
            S.activation(mu2[:], murow[:], AF.Square)
            var = T([1, 512], "ln_var", F32, bufs=1)
            V.scalar_tensor_tensor(var[:], ssum[:], 1.0 / 128, mu2[:],
                                   op0=ALU.mult, op1=ALU.subtract)
            lnv = T([1, 512], "ln_lnv", F32, bufs=1)
            S.activation(lnv[:], var[:], AF.Ln, bias=epscol[0:1, 0:1])
            rstd = T([1, 512], "ln_rstd", BF16)
            S.activation(rstd[:], lnv[:], AF.Exp, scale=-0.5)
            V.memset(lnrhs[:], 1.0)
            V.tensor_tensor(lnrhs[0:1, :], murow[:], rstd[:], op=ALU.mult)
            Rp = PA()
            MM(Rp[:], wt['lnwb'][:, lnidx, :], lnrhs[:], start=True, stop=True)
            rstdB = PA()
            MM(rstdB[:], onesrowB, rstd[:], start=True, stop=True)
            wcol = col(("lnw_spa0", "lnw_spa1", "lnw_norm")[lnidx])
            tmp = T([128, 512], "ln_tmp")
            V.tensor_tensor(tmp[:], xflat, rstdB[:], op=ALU.mult)
            if len(dst.shape) == 3:
                V.scalar_tensor_tensor(
                    dst, tmp[:].rearrange("p (s t) -> p s t", s=2), wcol,
                    Rp[:].rearrange("p (s t) -> p s t", s=2),
                    op0=ALU.mult, op1=ALU.add)
            else:
                V.scalar_tensor_tensor(dst, tmp[:], wcol, Rp[:],
                                       op0=ALU.mult, op1=ALU.add)

        # ================= spa mamba =================
        def spa_mamba(i, xs):
            xlnp = T([128, BPC, 259], "xlnp")
            V.memset(xlnp[:, :, 0:3], 0.0)
            part_ln(xs[:], i, xlnp[:, :, 3:259])
            xln = xlnp[:, :, 3:259]
            zsil = T([128, 2, 512], "mb_zsil")
            for j in range(2):
                pz = PA()
                MM(pz[:], wt['spa_z_w'][:, i, j * 128:(j + 1) * 128], xln,
                   start=True, stop=True)
                S.activation(zsil[:, j, :], pz[:], AF.Silu)
            # xBC blocks with conv folded: x halves + B + C
            xcx = T([128, 2, 2, 256], "mb_xcx")     # [p, j, s, t]
            xcB = T([64, 2, 256], "mb_xcB")
            xcC = T([64, 2, 256], "mb_xcC")
            blocks = [(0, 128, xcx[:, 0, :, :], col(f"spa_cb{i}_0")),
                      (128, 128, xcx[:, 1, :, :], col(f"spa_cb{i}_1")),
                      (256, 64, xcB[:], col(f"spa_cbB{i}", 64)),
                      (320, 64, xcC[:], col(f"spa_cbC{i}", 64))]
            for c0, rows, dst, cb in blocks:
                px = PA((rows, 512))
                for k in range(4):
                    MM(px[:].rearrange("p (s t) -> p s t", s=2),
                       wt['spa_xc_w'][:, i, k, c0:c0 + rows],
                       xlnp[:, :, k:k + 256],
                       start=(k == 0), stop=(k == 3))
                S.activation(dst.rearrange("p s t -> p (s t)"), px[:],
                             AF.Silu, bias=cb)
            # dt path (fp32)
            pdt = PS((4, 512))
            MM(pdt[:], wt['spa_dt_w'][:, i, :], xln, start=True, stop=True)
            e1 = T([4, 512], "mb_e1", F32, bufs=1)
            S.activation(e1[:], pdt[:], AF.Exp, bias=col(f"spa_dtb{i}", 4))
            dtv = T([4, 512], "mb_dtv", F32, bufs=1)
            S.activation(dtv[:], e1[:], AF.Ln, bias=onecolF[0:4, 0:1])
            eA = T([4, 1], "spa_eA", F32)
            S.activation(eA[:], col(f"spa_alog{i}", 4), AF.Exp)
            dtA = T([4, 512], "mb_dtA", F32, bufs=1)
            V.tensor_scalar(dtA[:], dtv[:], eA[:, 0:1], -1.0,
                            op0=ALU.mult, op1=ALU.mult)
            acum = T([4, 512], "mb_acum", F32, bufs=1)
            for s in range(BPC):
                V.tensor_tensor_scan(acum[:, s * 256:(s + 1) * 256],
                                     dtA[:, s * 256:(s + 1) * 256],
                                     dtA[:, s * 256:(s + 1) * 256], 0.0,
                                     op0=ALU.add, op1=ALU.bypass)
            aflat = T([1, 2, 1024], "aflat", F32, bufs=1)
            for s in range(BPC):
                dma(aflat[0:1, s, :].rearrange("o (p f) -> o p f", p=4),
                    acum[:, s * 256:(s + 1) * 256])
            ynt = T([128, 2, 2, 256], "mb_ynt")     # [p, j, s, t]
            for s in range(BPC):
                # acum+dt transposed: [tok, st, 8] f32 (cols 0:4 acum, 4:8 dt)
                acdtT = T([128, 2, 8], "spa_acdtT", F32)
                for st in range(2):
                    csl = slice(s * 256 + st * 128, s * 256 + (st + 1) * 128)
                    ptr = PB((128, 8))
                    TR(ptr[:, 0:4], acum[:, csl], identF[0:4, 0:4])
                    TR(ptr[:, 4:8], dtv[:, csl], identF[0:4, 0:4])
                    S.copy(acdtT[:, st, :], ptr[:, 0:8])
                pb1 = PB()
                MM(pb1[:], onesF[:], aflat[:, s, 0:512], start=True, stop=True)
                pb2 = PB()
                MM(pb2[:], onesF[:], aflat[:, s, 512:1024], start=True, stop=True)
                Dt = T([128, 2, 4, 256], "ssd_Dt")
                for st in range(2):
                    for h in range(H1):
                        pbx = pb1 if h < 2 else pb2
                        V.scalar_tensor_tensor(
                            Dt[:, st, h, :], pbx[:, (h % 2) * 256:(h % 2 + 1) * 256],
                            acdtT[:, st, h:h + 1], ct['minf_spa'][:, st, :],
                            op0=ALU.subtract, op1=ALU.min)
                Et = T([128, 2, 4, 256], "ssd_Et")
                S.activation(Et[:].rearrange("p a h t -> p (a h t)"),
                             Dt[:].rearrange("p a h t -> p (a h t)"), AF.Exp)
                pm0l = []
                for st in range(2):
                    pm0 = PB((128, 256))
                    MM(pm0[:], xcB[:, s, st * 128:(st + 1) * 128],
                       xcC[:, s, :], start=True, stop=True)
                    pm0l.append(pm0)
                MT = T([128, 2, 4, 256], "ssd_MT")
                for st in range(2):
                    for h in range(H1):
                        V.scalar_tensor_tensor(
                            MT[:, st, h, :], Et[:, st, h, :],
                            acdtT[:, st, 4 + h:5 + h], pm0l[st][:],
                            op0=ALU.mult, op1=ALU.mult)
                # token-major xc
                xtm = T([128, 2, 2, 128], "spa_xtm")   # [tok, st, j, 128]
                for st in range(2):
                    for j in range(2):
                        ptr = PB((128, 128), BF16)
                        TR(ptr[:, 0:128],
                           xcx[:, j, s, st * 128:(st + 1) * 128], ident[:])
                        S.copy(xtm[:, st, j, :], ptr[:, 0:128])
                ypY = PA()
                for st in range(2):
                    for h in range(H1):
                        MM(ypY[(h % 2) * 64:(h % 2) * 64 + 64,
                               (h // 2) * 256:(h // 2 + 1) * 256],
                           xtm[:, st, h // 2, (h % 2) * 64:(h % 2) * 64 + 64],
                           MT[:, st, h, :],
                           start=(st == 0), stop=(st == 1),
                           tile_position=(0, (h % 2) * 64),
                           skip_group_check=True)
                ygt = T([128, 2, 256], "spa_ygt")
                for j in range(2):
                    y0 = T([128, 256], "spa_y0")
                    V.scalar_tensor_tensor(y0[:], xcx[:, j, s, :],
                                           col(f"spa_dpc{i}_{j}"),
                                           ypY[:, j * 256:(j + 1) * 256],
                                           op0=ALU.mult, op1=ALU.add)
                    V.tensor_tensor(ygt[:, j, :], y0[:],
                                    zsil[:, j, s * 256:(s + 1) * 256], op=ALU.mult)
                # gated RMS over d_inner (256)
                sqy = T([128, 2, 256], "mb_sqy")
                S.activation(sqy[:].rearrange("p j t -> p (j t)"),
                             ygt[:].rearrange("p j t -> p (j t)"), AF.Square)
                ssy = PS((1, 256))
                for j in range(2):
                    MM(ssy[:], onescolB, sqy[:, j, :],
                       start=(j == 0), stop=(j == 1))
                varr = T([1, 256], "rms_var", F32, bufs=1)
                V.tensor_scalar(varr[:], ssy[:], 1.0 / 256, EPS,
                                op0=ALU.mult, op1=ALU.add)
                rl = T([1, 256], "rms_rl", F32, bufs=1)
                S.activation(rl[:], varr[:], AF.Ln)
                rrow = T([1, 256], "rms_rrow", BF16)
                S.activation(rrow[:], rl[:], AF.Exp, scale=-0.5)
                rB = PB((128, 256))
                MM(rB[:], onesrowB, rrow[:], start=True, stop=True)
                for j in range(2):
                    V.scalar_tensor_tensor(ynt[:, j, s, :], ygt[:, j, :],
                                           col(f"spa_rwc{i}_{j}"),
                                           rB[:], op0=ALU.mult, op1=ALU.mult)
            pop = PA()
            for j in range(2):
                MM(pop[:].rearrange("p (s t) -> p s t", s=2),
                   wt['spa_out_pk'][:, i, j, :], ynt[:, j, :, :],
                   start=(j == 0), stop=(j == 1))
            h1 = T([128, 2, 256], "h1")
            V.tensor_tensor(h1[:].rearrange("p s t -> p (s t)"), pop[:],
                            xs[:].rearrange("p s t -> p (s t)"), op=ALU.add)
            return h1

        # ================= spe mamba =================
        def spe_mamba(i, h1):
            # LayerNorm over the 256 features (free dim), per sample
            mus = T([128, 2], "spe_mus", F32)
            V.tensor_reduce(mus[:], h1[:], axis=AX.X, op=ALU.add)
            sq2 = T([128, 512], "ln_sq")
            S.activation(sq2[:], h1[:].rearrange("p s t -> p (s t)"), AF.Square)
            ss2 = T([128, 2], "spe_ss2", F32)
            V.tensor_reduce(ss2[:], sq2[:].rearrange("p (s t) -> p s t", s=2),
                            axis=AX.X, op=ALU.add)
            mean = T([128, 2], "spe_mean", F32)
            V.tensor_scalar(mean[:], mus[:], 1.0 / 256, None, op0=ALU.mult)
            m2 = T([128, 2], "spe_m2", F32)
            V.tensor_tensor(m2[:], mean[:], mean[:], op=ALU.mult)
            var2 = T([128, 2], "spe_var", F32)
            V.scalar_tensor_tensor(var2[:], ss2[:], 1.0 / 256, m2[:],
                                   op0=ALU.mult, op1=ALU.subtract)
            l2t = T([128, 2], "spe_l2", F32)
            S.activation(l2t[:], var2[:], AF.Ln, bias=epscol[:, 0:1])
            rstd2 = T([128, 2], "spe_rstd", F32)
            S.activation(rstd2[:], l2t[:], AF.Exp, scale=-0.5)
            X2fp = T([128, 2, 2, 131], "x2fp")      # [feat, s, kc, 3+tok]
            V.memset(X2fp[:, :, :, 0:3], 0.0)
            for s in range(BPC):
                xn = T([128, 256], "spe_xn")
                V.tensor_scalar(xn[:], h1[:, s, :], mean[:, s:s + 1], rstd2[:, s:s + 1],
                                op0=ALU.subtract, op1=ALU.mult)
                u = T([128, 256], "spe_u")
                V.tensor_tensor(u[:], xn[:], wt['spe_ln_wB'][:, i, :], op=ALU.mult)
                xsn = T([128, 256], "spe_xsn")
                V.tensor_tensor(xsn[:], u[:], wt['spe_ln_bB'][:, i, :], op=ALU.add)
                for ft in range(2):
                    ptr = PB((128, 128), BF16)
                    TR(ptr[:, 0:128], xsn[:, ft * 128:(ft + 1) * 128], ident[:])
                    S.copy(X2fp[:, s, ft, 3:131], ptr[:, 0:128])
            zsil = T([128, 2, 2, 2, 128], "mb_zsil")   # [p, g, jj, s, t]
            for g in range(2):
                pz = PA()
                for jj in range(2):
                    for kc in range(2):
                        MM(pz[:].rearrange("p (a s t) -> p a s t", a=2, s=2)[:, jj, :, :],
                           wt['spe_z_w'][:, i, kc,
                                         (2 * g + jj) * 128:(2 * g + jj + 1) * 128],
                           X2fp[:, :, kc, 3:131],
                           start=(kc == 0), stop=(kc == 1), skip_group_check=True)
                S.activation(zsil[:, g, :, :, :].rearrange("p a s t -> p (a s t)"),
                             pz[:], AF.Silu)
            xc2 = T([128, 4, 2, 128], "mb_xcx")     # [p, j, s, t]
            xcB = T([64, 2, 128], "mb_xcB")
            xcC = T([64, 2, 128], "mb_xcC")
            blocks = ([(blk * 128, 128, xc2[:, blk, :, :], col(f"spe_cb{i}_{blk}"))
                       for blk in range(4)]
                      + [(512, 64, xcB[:], col(f"spe_cbB{i}", 64)),
                         (576, 64, xcC[:], col(f"spe_cbC{i}", 64))])
            for c0, rows, dst, cb in blocks:
                px = PA((rows, 256))
                for k in range(4):
                    for kc in range(2):
                        MM(px[:].rearrange("p (s t) -> p s t", s=2),
                           wt['spe_xc_w'][:, i, k, kc, c0:c0 + rows],
                           X2fp[:, :, kc, k:k + 128],
                           start=(k == 0 and kc == 0), stop=(k == 3 and kc == 1))
                S.activation(dst.rearrange("p s t -> p (s t)"), px[:],
                             AF.Silu, bias=cb)
            # dt path (fp32)
            pdt = PS((8, 256))
            for kc in range(2):
                MM(pdt[:], wt['spe_dt_w'][:, i, kc, :], X2fp[:, :, kc, 3:131],
                   start=(kc == 0), stop=(kc == 1))
            e1 = T([8, 256], "mb_e1", F32, bufs=1)
            S.activation(e1[:], pdt[:], AF.Exp, bias=col(f"spe_dtb{i}", 8))
            dtv = T([8, 256], "mb_dtv", F32, bufs=1)
            S.activation(dtv[:], e1[:], AF.Ln, bias=onecolF[0:8, 0:1])
            eA = T([8, 1], "spe_eA", F32)
            S.activation(eA[:], col(f"spe_alog{i}", 8), AF.Exp)
            dtA = T([8, 256], "mb_dtA", F32, bufs=1)
            V.tensor_scalar(dtA[:], dtv[:], eA[:, 0:1], -1.0,
                            op0=ALU.mult, op1=ALU.mult)
            acum = T([8, 256], "mb_acum", F32, bufs=1)
            for s in range(BPC):
                V.tensor_tensor_scan(acum[:, s * 128:(s + 1) * 128],
                                     dtA[:, s * 128:(s + 1) * 128],
                                     dtA[:, s * 128:(s + 1) * 128], 0.0,
                                     op0=ALU.add, op1=ALU.bypass)
            aflat = T([1, 2, 1024], "aflat", F32, bufs=1)
            for s in range(BPC):
                dma(aflat[0:1, s, :].rearrange("o (p f) -> o p f", p=8),
                    acum[:, s * 128:(s + 1) * 128])
            ynt = T([128, 4, 2, 128], "mb_ynt")     # [p, j, s, t]
            for s in range(BPC):
                acdtT = T([128, 16], "spe_acdtT", F32)
                csl = slice(s * 128, (s + 1) * 128)
                ptr = PB((128, 16))
                TR(ptr[:, 0:8], acum[:, csl], identF[0:8, 0:8])
                TR(ptr[:, 8:16], dtv[:, csl], identF[0:8, 0:8])
                S.copy(acdtT[:], ptr[:, 0:16])
                pb1 = PB()
                MM(pb1[:], onesF[:], aflat[:, s, 0:512], start=True, stop=True)
                pb2 = PB()
                MM(pb2[:], onesF[:], aflat[:, s, 512:1024], start=True, stop=True)
                Dt = T([128, 8, 128], "ssd_Dt")
                for h in range(H2):
                    pbx = pb1 if h < 4 else pb2
                    V.scalar_tensor_tensor(
                        Dt[:, h, :], pbx[:, (h % 4) * 128:(h % 4 + 1) * 128],
                        acdtT[:, h:h + 1], ct['minf_spe'][:],
                        op0=ALU.subtract, op1=ALU.min)
                Et = T([128, 8, 128], "ssd_Et")
                S.activation(Et[:].rearrange("p h t -> p (h t)"),
                             Dt[:].rearrange("p h t -> p (h t)"), AF.Exp)
                pm0 = PB((128, 128))
                MM(pm0[:], xcB[:, s, :], xcC[:, s, :],
                   start=True, stop=True)
                MT = T([128, 8, 128], "ssd_MT")
                for h in range(H2):
                    V.scalar_tensor_tensor(
                        MT[:, h, :], Et[:, h, :], acdtT[:, 8 + h:9 + h], pm0[:],
                        op0=ALU.mult, op1=ALU.mult)
                xtm2 = T([128, 4, 128], "spa_xtm")  # [tok, j, 128]
                for j in range(4):
                    ptr = PB((128, 128), BF16)
                    TR(ptr[:, 0:128], xc2[:, j, s, :], ident[:])
                    S.copy(xtm2[:, j, :], ptr[:, 0:128])
                ypY = PA()
                for j in range(4):
                    for hh in range(2):
                        MM(ypY[hh * 64:hh * 64 + 64, j * 128:(j + 1) * 128],
                           xtm2[:, j, hh * 64:hh * 64 + 64],
                           MT[:, 2 * j + hh, :], start=True, stop=True,
                           tile_position=(0, hh * 64),
                           skip_group_check=True)
                ygt = T([128, 4, 128], "spe_ygt")
                for j in range(4):
                    y0 = T([128, 128], "spe_y0")
                    V.scalar_tensor_tensor(y0[:], xc2[:, j, s, :],
                                           col(f"spe_dpc{i}_{j}"),
                                           ypY[:, j * 128:(j + 1) * 128],
                                           op0=ALU.mult, op1=ALU.add)
                    V.tensor_tensor(ygt[:, j, :], y0[:],
                                    zsil[:, j // 2, j % 2, s, :], op=ALU.mult)
                sqy = T([128, 4, 128], "mb_sqy")
                S.activation(sqy[:].rearrange("p j t -> p (j t)"),
                             ygt[:].rearrange("p j t -> p (j t)"), AF.Square)
                ssy = PS((1, 128))
                for j in range(4):
                    MM(ssy[:], onescolB, sqy[:, j, :],
                       start=(j == 0), stop=(j == 3))
                varr = T([1, 128], "rms_var", F32, bufs=1)
                V.tensor_scalar(varr[:], ssy[:], 1.0 / 512, EPS,
                                op0=ALU.mult, op1=ALU.add)
                rl = T([1, 128], "rms_rl", F32, bufs=1)
                S.activation(rl[:], varr[:], AF.Ln)
                rrow = T([1, 128], "rms_rrow", BF16)
                S.activation(rrow[:], rl[:], AF.Exp, scale=-0.5)
                rB = PB((128, 128))
                MM(rB[:], onesrowB, rrow[:], start=True, stop=True)
                for j in range(4):
                    V.scalar_tensor_tensor(ynt[:, j, s, :], ygt[:, j, :],
                                           col(f"spe_rwc{i}_{j}"),
                                           rB[:], op0=ALU.mult, op1=ALU.mult)
            xs_new = T([128, 2, 256], "xs")
            for ft in range(2):
                ph2 = PB((128, 256))
                for k in range(4):
                    MM(ph2[:].rearrange("p (s t) -> p s t", s=2),
                       wt['spe_out_pk'][:, i, k, ft * 128:(ft + 1) * 128],
                       ynt[:, k, :, :], start=(k == 0), stop=(k == 3))
                h2f = T([128, 256], "spe_h2f")
                S.copy(h2f[:], ph2[:])
                for s in range(BPC):
                    ptr = PB((128, 128), BF16)
                    TR(ptr[:, 0:128], h2f[:, s * 128:(s + 1) * 128], ident[:])
                    V.tensor_tensor(xs_new[:, s, ft * 128:(ft + 1) * 128],
                                    ptr[:, 0:128], h1[:, s, ft * 128:(ft + 1) * 128],
                                    op=ALU.add)
            return xs_new

        # ================= layers =================
        cur = xs
        for i in range(2):
            h1 = spa_mamba(i, cur)
            tap(f"h1_{i}", h1[:].rearrange("p s t -> p (s t)"), (128, 512))
            cur = spe_mamba(i, h1)
            tap(f"xsl{i + 1}", cur[:].rearrange("p s t -> p (s t)"), (128, 512))

        # ================= final LN =================
        xf = T([128, 2, 256], "xf")
        part_ln(cur[:], 2, xf[:])
        xfl = xf[:].rearrange("p s t -> p (s t)")
        tap("xf", xfl, (128, 512))

        # ================= spa attention (center query) =================
        pctr = PS((128, 2))
        for l in range(5):
            MM(pctr[:], wt['cprj_pk'][:, l, :], xf[:, :, l],
               start=(l == 0), stop=(l == 4))
        ctr = T([128, 2], "at_ctr")
        S.activation(ctr[:], pctr[:], AF.Identity, bias=col("cprj_b"))
        pq = PS((128, 2))
        MM(pq[:], wt['aqT'][:], ctr[:], start=True, stop=True)
        qsb = T([128, 2], "at_q")
        S.activation(qsb[:], pq[:], AF.Identity, bias=col("aq_b"))
        pk = PA()
        MM(pk[:], wt['akT'][:], xfl, start=True, stop=True)
        Ksb = T([128, 2, 256], "at_K")
        S.activation(Ksb[:].rearrange("p s t -> p (s t)"), pk[:], AF.Identity,
                     bias=col("ak_b"))
        pv = PA()
        MM(pv[:], wt['avT'][:], xfl, start=True, stop=True)
        Vsb = T([128, 2, 256], "at_V")
        S.activation(Vsb[:].rearrange("p s t -> p (s t)"), pv[:], AF.Identity,
                     bias=col("av_b"))
        vo = T([128, 2, 256], "at_vo")
        for s in range(BPC):
            qd = T([128, 8], "at_qd")
            V.tensor_tensor(qd[:], qsb[:, s:s + 1].to_broadcast((128, 8)),
                            ct['Emask_q'][:], op=ALU.mult)
            plg = PS((8, 256))
            MM(plg[:], qd[:], Ksb[:, s, :], start=True, stop=True)
            nm = T([8, 1], "at_nm", F32)
            V.tensor_reduce(nm[:], plg[:], axis=AX.X, op=ALU.max, negate=True)
            nm4 = T([8, 1], "at_nm4", F32)
            V.tensor_scalar(nm4[:], nm[:], 0.25, None, op0=ALU.mult)
            ex = T([8, 256], "at_ex")
            S.activation(ex[:], plg[:], AF.Exp, bias=nm4[:, 0:1], scale=0.25)
            sm = T([8, 1], "at_sm", F32)
            V.tensor_reduce(sm[:], ex[:], axis=AX.X, op=ALU.add)
            rc = T([8, 1], "at_rc", F32)
            V.reciprocal(rc[:], sm[:])
            aw = T([8, 256], "at_aw")
            V.tensor_scalar(aw[:], ex[:], rc[:, 0:1], None, op0=ALU.mult)
            patB = PB((128, 256))
            MM(patB[:], ct['E_attn'][:], aw[:], start=True, stop=True)
            V.tensor_tensor(vo[:, s, :], Vsb[:, s, :], patB[:], op=ALU.mult)
        pao = PA()
        MM(pao[:], wt['aoT'][:], vo[:].rearrange("p s t -> p (s t)"),
           start=True, stop=True)
        xa = T([128, 2, 256], "xa")
        V.scalar_tensor_tensor(xa[:].rearrange("p s t -> p (s t)"), pao[:],
                               col("ao_b"), xfl, op0=ALU.add, op1=ALU.add)
        tap("xa", xa[:].rearrange("p s t -> p (s t)"), (128, 512))

        # ================= spe attention =================
        X2a = T([128, 2, 2, 128], "x2fp")
        for s in range(BPC):
            for ft in range(2):
                ptr = PB((128, 128), BF16)
                TR(ptr[:, 0:128], xa[:, s, ft * 128:(ft + 1) * 128], ident[:])
                S.copy(X2a[:, s, ft, :], ptr[:, 0:128])
        q2 = T([128, 2, 2, 128], "sp2_q2")   # [p, ot, s, t]
        k2 = T([128, 2, 2, 128], "sp2_k2")
        for ot in range(2):
            pq2 = PB((128, 256))
            for ft in range(2):
                MM(pq2[:].rearrange("p (s t) -> p s t", s=2),
                   wt['sqT'][:, ft, ot * 128:(ot + 1) * 128],
                   X2a[:, :, ft, :], start=(ft == 0), stop=(ft == 1))
            S.activation(q2[:, ot, :, :].rearrange("p s t -> p (s t)"),
                         pq2[:], AF.Identity, bias=col(f"sq_b{ot}"))
            pk2 = PB((128, 256))
            for ft in range(2):
                MM(pk2[:].rearrange("p (s t) -> p s t", s=2),
                   wt['skT'][:, ft, ot * 128:(ot + 1) * 128],
                   X2a[:, :, ft, :], start=(ft == 0), stop=(ft == 1))
            S.activation(k2[:, ot, :, :].rearrange("p s t -> p (s t)"),
                         pk2[:], AF.Identity, bias=col(f"sk_b{ot}"))
        xs2 = T([128, 2, 256], "xs2")
        for s in range(BPC):
            pv2 = PB((128, 256))
            for ft in range(2):
                MM(pv2[:], X2a[:, s, ft, :], wt['svT'][:, ft, :],
                   start=(ft == 0), stop=(ft == 1))
            v2 = T([128, 256], "sp2_v2")
            V.tensor_tensor(v2[:], pv2[:], wt['svbB'][:], op=ALU.add)
            pa2 = PB((128, 128))
            for ot in range(2):
                MM(pa2[:, 0:128], q2[:, ot, s, :], k2[:, ot, s, :],
                   start=(ot == 0), stop=(ot == 1))
            nm = T([128, 1], "sp2_nm", F32)
            V.tensor_reduce(nm[:], pa2[:, 0:128], axis=AX.X, op=ALU.max, negate=True)
            nm16 = T([128, 1], "sp2_nm16", F32)
            V.tensor_scalar(nm16[:], nm[:], 1.0 / 16, None, op0=ALU.mult)
            ex = T([128, 128], "sp2_ex")
            S.activation(ex[:], pa2[:, 0:128], AF.Exp, bias=nm16[:, 0:1], scale=1.0 / 16)
            sm = T([128, 1], "sp2_sm", F32)
            V.tensor_reduce(sm[:], ex[:], axis=AX.X, op=ALU.add)
            rc = T([128, 1], "sp2_rc", F32)
            V.reciprocal(rc[:], sm[:])
            a2 = T([128, 128], "sp2_a2")
            V.tensor_scalar(a2[:], ex[:], rc[:, 0:1], None, op0=ALU.mult)
            pa2T = PB((128, 128), BF16)
            TR(pa2T[:, 0:128], a2[:], ident[:])
            a2T = T([128, 128], "sp2_a2T")
            S.copy(a2T[:], pa2T[:, 0:128])
            o2 = T([128, 2, 128], "sp2_o2")
            for ot in range(2):
                po2 = PB((128, 128))
                MM(po2[:, 0:128], v2[:, ot * 128:(ot + 1) * 128], a2T[:],
                   start=True, stop=True)
                S.copy(o2[:, ot, :], po2[:, 0:128])
            po3 = PB((128, 256))
            for ot in range(2):
                MM(po3[:], o2[:, ot, :], wt['soT'][:, ot, :],
                   start=(ot == 0), stop=(ot == 1))
            t3 = T([128, 256], "sp2_t3")
            V.tensor_tensor(t3[:], po3[:], wt['sobB'][:], op=ALU.add)
            V.tensor_tensor(xs2[:, s, :], t3[:], xa[:, s, :], op=ALU.add)
        tap("xs2", xs2[:].rearrange("p s t -> p (s t)"), (128, 512))

        # ================= downsample =================
        invr = T([1, BPC, L], "irow_raw", I32, bufs=1)
        dma(invr[:], inv[None, :, :])
        invf = T([1, BPC, L], "irow_f", F32, bufs=1)
        V.tensor_copy(invf[:], invr[:])
        pds = psD.tile([64, 512], F32, tag="ds", name="pds")
        for s in range(BPC):
            invB = PB()
            MM(invB[:, 0:L], onesF[:], invf[:, s, :], start=True, stop=True)
            QT = T([128, 2, 256], "perm_oh")
            for tt in range(2):
                V.tensor_scalar(QT[:, tt, :], invB[:, 0:L], ct['iotaC'][:, tt:tt + 1],
                                None, op0=ALU.is_equal)
            tmv = T([128, 2, 128], "tm_tmp")
            for tt in range(2):
                ptr = PB((128, 128), BF16)
                TR(ptr[:, 0:128], xs2[:, s, tt * 128:(tt + 1) * 128], ident[:])
                S.copy(tmv[:, tt, :], ptr[:, 0:128])
            pxr = PB((128, 256))
            for tt in range(2):
                MM(pxr[:], tmv[:, tt, :], QT[:, tt, :],
                   start=(tt == 0), stop=(tt == 1))
            xrp = T([128, 324], "ds_xrp")
            V.memset(xrp[:], 0.0)
            xr3 = xrp[:].rearrange("p (h w) -> p h w", h=18)
            S.copy(xr3[:, 1:17, 1:17], pxr[:].rearrange("p (h w) -> p h w", h=16))
            for kh in range(3):
                for kw in range(3):
                    k = kh * 3 + kw
                    cmp_ = T([128, 64], "ds_cmp")
                    V.tensor_copy(cmp_[:].rearrange("p (a b) -> p a b", a=8),
                                  xr3[:, kh:kh + 16:2, kw:kw + 16:2])
                    MM(pds[:, s * 128:(s + 1) * 128],
                       cmp_[:],
                       wt['dsw_pk'][:, k, :],
                       start=(k == 0), stop=(k == 8),
                       skip_group_check=True)
        for s in range(BPC):
            view = pds[:, s * 128:(s + 1) * 128]
            mus = T([64, 1], "ds_mus", F32)
            V.tensor_reduce(mus[:], view, axis=AX.X, op=ALU.add)
            mean = T([64, 1], "ds_mean", F32)
            V.tensor_scalar(mean[:], mus[:], 1.0 / 128, None, op0=ALU.mult)
            sq = T([64, 128], "ds_sq", F32)
            S.activation(sq[:], view, AF.Square)
            ss = T([64, 1], "ds_ss", F32)
            V.tensor_reduce(ss[:], sq[:], axis=AX.X, op=ALU.add)
            m2 = T([64, 1], "ds_m2", F32)
            V.tensor_tensor(m2[:], mean[:], mean[:], op=ALU.mult)
            var = T([64, 1], "ds_var", F32)
            V.scalar_tensor_tensor(var[:], ss[:], 1.0 / 128, m2[:],
                                   op0=ALU.mult, op1=ALU.subtract)
            lv = T([64, 1], "ds_lv", F32)
            S.activation(lv[:], var[:], AF.Ln, bias=epscol[0:64, 0:1])
            rstd = T([64, 1], "ds_rstd", F32)
            S.activation(rstd[:], lv[:], AF.Exp, scale=-0.5)
            xn = T([64, 128], "ds_xn", F32)
            V.tensor_scalar(xn[:], view, mean[:, 0:1], rstd[:, 0:1],
                            op0=ALU.subtract, op1=ALU.mult)
            t1 = T([64, 128], "ds_t1", F32)
            V.tensor_tensor(t1[:], xn[:], wt['ds_ln_wB'][:], op=ALU.mult)
            o1 = T([64, 128], "ds_o1", F32)
            V.tensor_tensor(o1[:], t1[:], wt['ds_ln_bB'][:], op=ALU.add)
            dma(out[s].rearrange("h w c -> (h w) c"), o1[:])

        stk.close()
    return nc, tap_t


# ---------------------------------------------------------------------------
_CACHE = {}


def _get_program(taps=()):
    key = tuple(sorted(taps))
    if key not in _CACHE:
        _CACHE[key] = build_program(taps)
    return _CACHE[key]


def make_inmaps(inputs, taps=()):
    cst = host_constants()
    w = prep_weights(inputs)
    x = np.asarray(inputs['x'], np.float32).reshape(16, C, L).astype(NPBF)
    idx = np.asarray(inputs['sorted_index'], np.int32)
    inv = np.argsort(idx, axis=1, kind='stable').astype(np.int32)
    in_maps = []
    for c in range(NCORES):
        m = {}
        m.update({k: np.ascontiguousarray(v) for k, v in cst.items()})
        m.update({k: np.ascontiguousarray(v) for k, v in w.items()})
        sl = slice(c * BPC, (c + 1) * BPC)
        m['x2'] = np.ascontiguousarray(x[sl])
        m['idx'] = np.ascontiguousarray(idx[sl])
        m['inv'] = np.ascontiguousarray(inv[sl])
        in_maps.append(m)
    return in_maps


def run(inputs, taps=(), trace=False):
    nc, tap_t = _get_program(taps)
    in_maps = make_inmaps(inputs, taps)
    res = run_bass_kernel_spmd(nc, in_maps, list(range(NCORES)), trace=trace)
    outs = np.concatenate([np.asarray(r['out'], np.float32) for r in res.results],
                          axis=0)
    tapd = {}
    for name in taps:
        tapd[name] = [np.asarray(r.get('t_' + name), np.float32)
                      for r in res.results]
    return outs, tapd, res


def kernel(**inputs):
    outs, _, _ = run(inputs)
    return outs



# revision 46
# speedup vs baseline: 1.4239x; 1.0739x over previous
"""Trainium2 Bass kernel for nn_Basic_Block_v1 (spatial/spectral Mamba2 block).

Sharding: data-parallel over batch (16 samples) across 8 NeuronCores,
2 samples per core; all parameters replicated. Heavy math in bf16 on the
TensorEngine (1 cyc/row vs 4 for fp32); the SSD decay cumsum path stays fp32.
Depthwise convs are folded into the in_proj matmuls via host-side weight
scaling with shifted moving operands (zero-padded token axis).
"""
import sys
sys.path.insert(0, '/opt/trn_rl_repo')
import json

import numpy as np
import ml_dtypes

import concourse.bass as bass
import concourse.mybir as mybir
from concourse import tile
from concourse.bass_utils import run_bass_kernel_spmd

F32 = mybir.dt.float32
BF16 = mybir.dt.bfloat16
I32 = mybir.dt.int32
AF = mybir.ActivationFunctionType
ALU = mybir.AluOpType
AX = mybir.AxisListType
NPBF = ml_dtypes.bfloat16

NCORES = 8
BPC = 2          # batch per core
L = 256          # spatial tokens
C = 128          # channels
H1 = 4           # spa heads
H2 = 8           # spe heads
EPS = 1e-5
NEG = -88.0

# ---------------------------------------------------------------------------
# walrus in this container supports only ONE sync-wait per instruction;
# split extra waits emitted by the Tile scheduler onto preceding NoOps.
_WAIT_LIMIT = 1
_orig_to_json = bass.Bass.to_json_bytes


def _fix_block(b, ctr):
    insts = b.get('instructions')
    if insts:
        out = []
        for ins in insts:
            si = ins.get('sync_info')
            waits = (si or {}).get('on_wait') or []
            if len(waits) > _WAIT_LIMIT:
                while len(waits) > _WAIT_LIMIT:
                    chunk, waits = waits[:_WAIT_LIMIT], waits[_WAIT_LIMIT:]
                    ctr[0] += 1
                    out.append({
                        "debug": ins.get("debug"),
                        "engine": ins["engine"],
                        "ins": [],
                        "name": f"I-wsplit{ctr[0]}",
                        "opcode": "NoOp",
                        "outs": [],
                        "text_hint": "wsplit",
                        "sync_info": {"on_update": [], "on_wait": chunk},
                    })
                si['on_wait'] = waits
            out.append(ins)
        b['instructions'] = out
    for sb in b.get('blocks') or []:
        _fix_block(sb, ctr)


def _patched_to_json(self, *a, **k):
    raw = _orig_to_json(self, *a, **k)
    d = json.loads(raw)
    ctr = [0]
    for f in d.get('functions', []):
        for b in f.get('blocks', []):
            _fix_block(b, ctr)
    if ctr[0] == 0:
        return raw
    return json.dumps(d).encode()


bass.Bass.to_json_bytes = _patched_to_json


# ---------------------------------------------------------------------------
def _sincos_2d(dim, Hg):
    def e1(d, pos):
        omega = 1.0 / (10000.0 ** (np.arange(d // 2, dtype=np.float64) / (d / 2.0)))
        out = pos[:, None] * omega[None, :]
        return np.concatenate([np.sin(out), np.cos(out)], axis=-1)
    gh, gw = np.meshgrid(np.arange(Hg), np.arange(Hg), indexing='ij')
    emb = np.concatenate([e1(dim // 2, gh.reshape(-1)), e1(dim // 2, gw.reshape(-1))], axis=-1)
    return emb.astype(np.float32)


def host_constants():
    d = {}
    d['pe_fm'] = np.ascontiguousarray(_sincos_2d(C, 16).T).astype(NPBF)   # [128, 256]
    d['ident'] = np.eye(128, dtype=np.float32).astype(NPBF)
    d['identF'] = np.eye(16, dtype=np.float32)
    iota = np.arange(L, dtype=np.float32)
    d['iotaC'] = np.stack([iota[:128], iota[128:]], axis=1).copy()        # [128, 2] f32
    # Minf[sp][st][t] = 0 if (st*128+sp) <= t else NEG   (spa, L=256)
    sidx = np.arange(L)[:, None]
    tidx = np.arange(L)[None, :]
    m = np.where(sidx <= tidx, 0.0, NEG).astype(np.float32)               # [s, t]
    d['minf_spa'] = np.stack([m[:128], m[128:]], axis=1).astype(NPBF)     # [128, 2, 256]
    s2 = np.arange(C)[:, None]
    t2 = np.arange(C)[None, :]
    d['minf_spe'] = np.where(s2 <= t2, 0.0, NEG).astype(NPBF)             # [128, 128]
    EA = np.zeros((8, 128), np.float32)
    for h in range(8):
        EA[h, h * 16:(h + 1) * 16] = 1.0
    d['E_attn'] = EA.astype(NPBF)                                         # [8, 128]
    d['Emask_q'] = EA.T.copy().astype(NPBF)                               # [128, 8]
    return d


COL_ORDER = (
    [f"spa_dtb{i}" for i in range(2)] + [f"spa_alog{i}" for i in range(2)]
    + [f"spa_cb{i}_{b}" for i in range(2) for b in range(2)]
    + [f"spa_cbB{i}" for i in range(2)] + [f"spa_cbC{i}" for i in range(2)]
    + [f"spa_dpc{i}_{j}" for i in range(2) for j in range(2)]
    + [f"spa_rwc{i}_{j}" for i in range(2) for j in range(2)]
    + [f"spe_dtb{i}" for i in range(2)] + [f"spe_alog{i}" for i in range(2)]
    + [f"spe_cb{i}_{b}" for i in range(2) for b in range(4)]
    + [f"spe_cbB{i}" for i in range(2)] + [f"spe_cbC{i}" for i in range(2)]
    + [f"spe_dpc{i}_{j}" for i in range(2) for j in range(4)]
    + [f"spe_rwc{i}_{j}" for i in range(2) for j in range(4)]
    + ["lnw_spa0", "lnw_spa1", "lnw_norm",
       "cprj_b", "aq_b", "ak_b", "av_b", "ao_b",
       "sq_b0", "sq_b1", "sk_b0", "sk_b1"]
)
CIDX = {k: ix for ix, k in enumerate(COL_ORDER)}


def prep_weights(inp):
    """Host-side layout prep: bf16 casts, transposes, conv folding."""
    w = {}
    bf = lambda a: np.ascontiguousarray(a).astype(NPBF)
    # ---- spa mamba: in_w [2, 644, 128]; conv folded into xBC blocks ----
    spa_z = np.zeros((2, 128, 256), np.float32)
    spa_xc = np.zeros((2, 4, 128, 384), np.float32)
    spa_dt = np.zeros((2, 128, 4), np.float32)
    for i in range(2):
        W = np.asarray(inp['spa_in_w'][i], np.float32)          # [644, 128]
        cw = np.asarray(inp['spa_conv_w'][i], np.float32)       # [384, 4]
        spa_z[i] = W[0:256].T
        for k in range(4):
            spa_xc[i, k] = (W[256:640] * cw[:, k:k + 1]).T
        spa_dt[i] = W[640:644].T
    w['spa_z_w'] = bf(spa_z.transpose(1, 0, 2))          # [128, 2, 256]
    w['spa_xc_w'] = bf(spa_xc.transpose(2, 0, 1, 3))     # [128, 2, 4, 384]
    w['spa_dt_w'] = bf(spa_dt.transpose(1, 0, 2))        # [128, 2, 4]
    sow = np.transpose(inp['spa_out_w'], (0, 2, 1)).reshape(2, 2, 128, 128)
    w['spa_out_pk'] = bf(np.transpose(sow, (2, 0, 1, 3)))       # [128, 2, 2, 128]
    # ---- spe mamba: in_w [2, 1160, 256] ----
    spe_z = np.zeros((2, 2, 128, 512), np.float32)
    spe_xc = np.zeros((2, 4, 2, 128, 640), np.float32)
    spe_dt = np.zeros((2, 2, 128, 8), np.float32)
    for i in range(2):
        W = np.asarray(inp['spe_in_w'][i], np.float32)          # [1160, 256]
        cw = np.asarray(inp['spe_conv_w'][i], np.float32)       # [640, 4]
        for kc in range(2):
            cs = slice(kc * 128, (kc + 1) * 128)
            spe_z[i, kc] = W[0:512, cs].T
            spe_dt[i, kc] = W[1152:1160, cs].T
            for k in range(4):
                spe_xc[i, k, kc] = (W[512:1152, cs] * cw[:, k:k + 1]).T
    w['spe_z_w'] = bf(spe_z.transpose(2, 0, 1, 3))       # [128, 2, 2, 512]
    w['spe_xc_w'] = bf(spe_xc.transpose(3, 0, 1, 2, 4))  # [128, 2, 4, 2, 640]
    w['spe_dt_w'] = bf(spe_dt.transpose(2, 0, 1, 3))     # [128, 2, 2, 8]
    sew = np.transpose(inp['spe_out_w'], (0, 2, 1)).reshape(2, 4, 128, 256)
    w['spe_out_pk'] = bf(np.transpose(sew, (2, 0, 1, 3)))       # [128, 2, 4, 256]
    w['spe_ln_wB'] = bf(np.broadcast_to(
        inp['spe_ln_w'][:, None, :], (2, 128, 256)).transpose(1, 0, 2))
    w['spe_ln_bB'] = bf(np.broadcast_to(
        inp['spe_ln_b'][:, None, :], (2, 128, 256)).transpose(1, 0, 2))
    # ---- attention / head ----
    w['cprj_pk'] = bf(np.transpose(inp['cprj_w'], (2, 1, 0)).transpose(1, 0, 2))
    for nm in ('aq', 'ak', 'av', 'ao'):
        w[nm + 'T'] = bf(inp[nm + '_w'].T)
    for nm in ('sq', 'sk', 'sv', 'so'):
        wt_ = np.asarray(inp[nm + '_w'], np.float32).T.reshape(2, 128, 256)
        w[nm + 'T'] = bf(wt_.transpose(1, 0, 2))
    w['svbB'] = bf(np.broadcast_to(inp['sv_b'][None, :], (128, 256)))
    w['sobB'] = bf(np.broadcast_to(inp['so_b'][None, :], (128, 256)))
    w['dsw_pk'] = bf(np.asarray(inp['ds_conv_w'], np.float32)
                     .reshape(9, 128, 128).transpose(1, 0, 2))
    w['ds_ln_wB'] = np.ascontiguousarray(
        np.broadcast_to(inp['ds_ln_w'][None, :], (64, 128))).astype(np.float32)
    w['ds_ln_bB'] = np.ascontiguousarray(
        np.broadcast_to(inp['ds_ln_b'][None, :], (64, 128))).astype(np.float32)
    # partition-dim layernorm stationary: row0 = -w (sign trick), row1 = b
    lnwb = np.zeros((2, 3, 128), np.float32)
    lnwb[0, 0], lnwb[1, 0] = -np.asarray(inp['spa_ln_w'][0]), inp['spa_ln_b'][0]
    lnwb[0, 1], lnwb[1, 1] = -np.asarray(inp['spa_ln_w'][1]), inp['spa_ln_b'][1]
    lnwb[0, 2], lnwb[1, 2] = -np.asarray(inp['norm_w']), inp['norm_b']
    w['lnwb'] = bf(lnwb)
    # ---- f32 scalar column pack ----
    cols = {}
    for i in range(2):
        cols[f"spa_dtb{i}"] = inp['spa_dt_bias'][i]
        cols[f"spa_alog{i}"] = inp['spa_A_log'][i]
        cb = np.asarray(inp['spa_conv_b'][i], np.float32)
        cols[f"spa_cb{i}_0"] = cb[0:128]
        cols[f"spa_cb{i}_1"] = cb[128:256]
        cols[f"spa_cbB{i}"] = cb[256:320]
        cols[f"spa_cbC{i}"] = cb[320:384]
        for j in range(2):
            cols[f"spa_dpc{i}_{j}"] = np.repeat(inp['spa_D'][i], 64)[j * 128:(j + 1) * 128]
            cols[f"spa_rwc{i}_{j}"] = inp['spa_rms_w'][i, j * 128:(j + 1) * 128]
        cols[f"spe_dtb{i}"] = inp['spe_dt_bias'][i]
        cols[f"spe_alog{i}"] = inp['spe_A_log'][i]
        cb2 = np.asarray(inp['spe_conv_b'][i], np.float32)
        for b in range(4):
            cols[f"spe_cb{i}_{b}"] = cb2[b * 128:(b + 1) * 128]
        cols[f"spe_cbB{i}"] = cb2[512:576]
        cols[f"spe_cbC{i}"] = cb2[576:640]
        for j in range(4):
            cols[f"spe_dpc{i}_{j}"] = np.repeat(inp['spe_D'][i], 64)[j * 128:(j + 1) * 128]
            cols[f"spe_rwc{i}_{j}"] = inp['spe_rms_w'][i, j * 128:(j + 1) * 128]
    cols["lnw_spa0"] = inp['spa_ln_w'][0]
    cols["lnw_spa1"] = inp['spa_ln_w'][1]
    cols["lnw_norm"] = inp['norm_w']
    cols["cprj_b"] = inp['cprj_b']
    for nm in ('aq', 'ak', 'av', 'ao'):
        cols[nm + "_b"] = inp[nm + '_b']
    cols["sq_b0"] = inp['sq_b'][0:128]
    cols["sq_b1"] = inp['sq_b'][128:256]
    cols["sk_b0"] = inp['sk_b'][0:128]
    cols["sk_b1"] = inp['sk_b'][128:256]
    pk = np.zeros((128, len(COL_ORDER)), np.float32)
    for k, v in cols.items():
        v = np.asarray(v, np.float32)
        pk[0:v.shape[0], CIDX[k]] = v
    w['colpak'] = pk
    return w


WSHAPES = {
    'spa_z_w': ([128, 2, 256], BF16), 'spa_xc_w': ([128, 2, 4, 384], BF16),
    'spa_dt_w': ([128, 2, 4], BF16), 'spa_out_pk': ([128, 2, 2, 128], BF16),
    'spe_z_w': ([128, 2, 2, 512], BF16), 'spe_xc_w': ([128, 2, 4, 2, 640], BF16),
    'spe_dt_w': ([128, 2, 2, 8], BF16), 'spe_out_pk': ([128, 2, 4, 256], BF16),
    'spe_ln_wB': ([128, 2, 256], BF16), 'spe_ln_bB': ([128, 2, 256], BF16),
    'cprj_pk': ([128, 5, 128], BF16),
    'aqT': ([128, 128], BF16), 'akT': ([128, 128], BF16),
    'avT': ([128, 128], BF16), 'aoT': ([128, 128], BF16),
    'sqT': ([128, 2, 256], BF16), 'skT': ([128, 2, 256], BF16),
    'svT': ([128, 2, 256], BF16), 'soT': ([128, 2, 256], BF16),
    'svbB': ([128, 256], BF16), 'sobB': ([128, 256], BF16),
    'dsw_pk': ([128, 9, 128], BF16),
    'ds_ln_wB': ([64, 128], F32), 'ds_ln_bB': ([64, 128], F32),
    'lnwb': ([2, 3, 128], BF16), 'colpak': ([128, len(COL_ORDER)], F32),
}
CSHAPES = {
    'pe_fm': ([128, 256], BF16), 'ident': ([128, 128], BF16),
    'identF': ([16, 16], F32), 'iotaC': ([128, 2], F32),
    'minf_spa': ([128, 2, 256], BF16), 'minf_spe': ([128, 128], BF16),
    'E_attn': ([8, 128], BF16), 'Emask_q': ([128, 8], BF16),
}

# ---------------------------------------------------------------------------
def build_program(taps=()):
    """Builds the per-core SPMD Bass program."""
    nc = bass.Bass()

    def din(name, shape, dt=F32):
        return nc.dram_tensor(name, shape, dt, kind="ExternalInput")

    x2 = din("x2", [BPC, C, L], BF16)
    idx = din("idx", [BPC, L], I32)
    inv = din("inv", [BPC, L], I32)
    cst_t = {k: din(k, shp, dt) for k, (shp, dt) in CSHAPES.items()}
    w_t = {k: din(k, shp, dt) for k, (shp, dt) in WSHAPES.items()}
    out = nc.dram_tensor("out", [BPC, 8, 8, C], F32, kind="ExternalOutput")
    tap_t = {}

    with tile.TileContext(nc) as tc:
        import contextlib
        stk = contextlib.ExitStack()
        sb = stk.enter_context(tc.tile_pool(name="sb", bufs=2))
        wb = stk.enter_context(tc.tile_pool(name="wb", bufs=1))
        psA = stk.enter_context(tc.tile_pool(name="psA", bufs=2, space="PSUM"))
        psB = stk.enter_context(tc.tile_pool(name="psB", bufs=3, space="PSUM"))
        psS = stk.enter_context(tc.tile_pool(name="psS", bufs=2, space="PSUM"))
        psD = stk.enter_context(tc.tile_pool(name="psD", bufs=1, space="PSUM"))

        def T(shape, tag, dt=BF16, bufs=None):
            return sb.tile(shape, dt, tag=tag, name=tag, bufs=bufs)

        def W(shape, tag, dt=BF16):
            return wb.tile(shape, dt, tag=tag, name=tag, bufs=1)

        def PA(shape=(128, 512), dt=F32):
            return psA.tile(list(shape), dt, tag="A", name="pa",
                            padded_shape=[128, 512 if dt == F32 else 1024])

        def PB(shape=(128, 512), dt=F32):
            return psB.tile(list(shape), dt, tag="B", name="pb",
                            padded_shape=[128, 512 if dt == F32 else 1024])

        def PS(shape=(16, 512), dt=F32):
            return psS.tile(list(shape), dt, tag="S", name="ps",
                            padded_shape=[shape[0], 512])

        dma = nc.sync.dma_start
        V = nc.vector
        S = nc.scalar
        G = nc.gpsimd
        MM = nc.tensor.matmul
        TR = nc.tensor.transpose

        # ---------- load constants + weights (single DMA per tensor) ----------
        ct = {}
        for k, (shp, dt) in CSHAPES.items():
            ct[k] = W(shp, "c_" + k, dt)
            dma(ct[k][:], cst_t[k][:])
        wt = {}
        for k, (shp, dt) in WSHAPES.items():
            wt[k] = W(shp, "w_" + k, dt)
            dma(wt[k][:], w_t[k][:])
        colpak = wt['colpak']

        def col(key, p=128):
            return colpak[0:p, CIDX[key]:CIDX[key] + 1]

        onesB = W([128, 128], "onesB", BF16)
        V.memset(onesB[:], 1.0)
        onescolB = onesB[:, 0:1]
        onesrowB = onesB[0:1, :]
        onesF = W([1, 128], "onesF", F32)
        V.memset(onesF[:], 1.0)
        onecolF = W([128, 1], "onecolF", F32)
        V.memset(onecolF[:], 1.0)
        epscol = W([128, 1], "epscol", F32)
        V.memset(epscol[:], EPS)
        ident = ct['ident']
        identF = ct['identF']

        # layer-constant A-exponentials
        eA_spa = W([4, 2], "eA_spa", F32)
        eA_spe = W([8, 2], "eA_spe", F32)
        for i in range(2):
            S.activation(eA_spa[:, i:i + 1], col(f"spa_alog{i}", 4), AF.Exp)
            S.activation(eA_spe[:, i:i + 1], col(f"spe_alog{i}", 8), AF.Exp)

        def tap(name, src_ap, shape):
            # stage through f32 + DMA out (debug only)
            if name not in taps:
                return
            st_ = T(list(shape), "tapstage", F32)
            S.copy(st_[:], src_ap)
            t = nc.dram_tensor("t_" + name, list(shape), F32, kind="ExternalOutput")
            tap_t[name] = t
            dma(t[:], st_[:])

        # ---------- stage 0: embed + permute ----------
        xb = T([128, BPC, L], "xb")
        for s in range(BPC):
            dma(xb[:, s, :], x2[s])
        x0 = T([128, BPC, L], "x0")
        V.tensor_tensor(
            x0[:], xb[:],
            ct['pe_fm'][:].unsqueeze(1).to_broadcast((128, BPC, L)),
            op=ALU.add)

        idxr = T([1, BPC, L], "irow_raw", I32, bufs=1)
        dma(idxr[:], idx[None, :, :])
        idxf = T([1, BPC, L], "irow_f", F32, bufs=1)
        V.tensor_copy(idxf[:], idxr[:])

        xs = T([128, BPC, L], "xs")
        for s in range(BPC):
            idxB = PB()
            MM(idxB[:, 0:L], onesF[:], idxf[:, s, :], start=True, stop=True)
            PmT = T([128, 2, L], "perm_oh")
            for st in range(2):
                V.tensor_scalar(PmT[:, st, :], idxB[:, 0:L], ct['iotaC'][:, st:st + 1],
                                None, op0=ALU.is_equal)
            x0tm = T([128, 2, 128], "tm_tmp")
            for tt in range(2):
                ptr = PB((128, 128), BF16)
                TR(ptr[:, 0:128], x0[:, s, tt * 128:(tt + 1) * 128], ident[:])
                S.copy(x0tm[:, tt, :], ptr[:, 0:128])
            pxs = PB((128, 256))
            for st in range(2):
                MM(pxs[:], x0tm[:, st, :], PmT[:, st, :],
                   start=(st == 0), stop=(st == 1))
            S.copy(xs[:, s, :], pxs[:])
        tap("xs0", xs[:].rearrange("p s t -> p (s t)"), (128, 512))

        # ================= partition-dim layernorm =================
        lnrhs = T([2, 512], "ln_rhs", BF16, bufs=2)

        def part_ln(xsrc, lnidx, dst):
            """LN over channel (partition) dim. xsrc/dst: [128, 2, 256] views."""
            xflat = xsrc.rearrange("p s t -> p (s t)") if len(xsrc.shape) == 3 else xsrc
            sq = T([128, 512], "ln_sq")
            S.activation(sq[:], xflat, AF.Square)
            msum = PS((1, 512))
            MM(msum[:], onescolB, xflat, start=True, stop=True)
            ssum = PS((1, 512))
            MM(ssum[:], onescolB, sq[:], start=True, stop=True)
            murow = T([1, 512], "ln_mu", F32, bufs=1)
            S.activation(murow[:], msum[:], AF.Copy, scale=1.0 / 128)
            mu2 = T([1, 512], "ln_mu2", F32, bufs=1)
            S.activation(mu2[:], murow[:], AF.Square)
            var = T([1, 512], "ln_var", F32, bufs=1)
            V.scalar_tensor_tensor(var[:], ssum[:], 1.0 / 128, mu2[:],
                                   op0=ALU.mult, op1=ALU.subtract)
            lnv = T([1, 512], "ln_lnv", F32, bufs=1)
            S.activation(lnv[:], var[:], AF.Ln, bias=epscol[0:1, 0:1])
            rstd = T([1, 512], "ln_rstd", BF16)
            S.activation(rstd[:], lnv[:], AF.Exp, scale=-0.5)
            V.memset(lnrhs[:], 1.0)
            V.tensor_tensor(lnrhs[0:1, :], murow[:], rstd[:], op=ALU.mult)
            Rp = PA()
            MM(Rp[:], wt['lnwb'][:, lnidx, :], lnrhs[:], start=True, stop=True)
            rstdB = PA()
            MM(rstdB[:], onesrowB, rstd[:], start=True, stop=True)
            wcol = col(("lnw_spa0", "lnw_spa1", "lnw_norm")[lnidx])
            tmp = T([128, 512], "ln_tmp")
            V.tensor_tensor(tmp[:], xflat, rstdB[:], op=ALU.mult)
            if len(dst.shape) == 3:
                V.scalar_tensor_tensor(
                    dst, tmp[:].rearrange("p (s t) -> p s t", s=2), wcol,
                    Rp[:].rearrange("p (s t) -> p s t", s=2),
                    op0=ALU.mult, op1=ALU.add)
            else:
                V.scalar_tensor_tensor(dst, tmp[:], wcol, Rp[:],
                                       op0=ALU.mult, op1=ALU.add)

        # ================= spa mamba =================
        def spa_mamba(i, xs):
            xlnp = T([128, BPC, 259], "xlnp")
            V.memset(xlnp[:, :, 0:3], 0.0)
            part_ln(xs[:], i, xlnp[:, :, 3:259])
            xln = xlnp[:, :, 3:259]
            zsil = T([128, 2, 512], "mb_zsil")
            for j in range(2):
                pz = PA()
                MM(pz[:], wt['spa_z_w'][:, i, j * 128:(j + 1) * 128], xln,
                   start=True, stop=True)
                S.activation(zsil[:, j, :], pz[:], AF.Silu)
            # xBC blocks with conv folded: x halves + B + C
            xcx = T([128, 2, 2, 256], "mb_xcx")     # [p, j, s, t]
            xcB = T([64, 2, 256], "mb_xcB")
            xcC = T([64, 2, 256], "mb_xcC")
            blocks = [(0, 128, xcx[:, 0, :, :], col(f"spa_cb{i}_0")),
                      (128, 128, xcx[:, 1, :, :], col(f"spa_cb{i}_1")),
                      (256, 64, xcB[:], col(f"spa_cbB{i}", 64)),
                      (320, 64, xcC[:], col(f"spa_cbC{i}", 64))]
            for c0, rows, dst, cb in blocks:
                px = PA((rows, 512))
                for k in range(4):
                    MM(px[:].rearrange("p (s t) -> p s t", s=2),
                       wt['spa_xc_w'][:, i, k, c0:c0 + rows],
                       xlnp[:, :, k:k + 256],
                       start=(k == 0), stop=(k == 3))
                S.activation(dst.rearrange("p s t -> p (s t)"), px[:],
                             AF.Silu, bias=cb)
            # dt path (fp32)
            pdt = PS((4, 512))
            MM(pdt[:], wt['spa_dt_w'][:, i, :], xln, start=True, stop=True)
            e1 = T([4, 512], "mb_e1", F32, bufs=1)
            S.activation(e1[:], pdt[:], AF.Exp, bias=col(f"spa_dtb{i}", 4))
            dtv = T([4, 512], "mb_dtv", F32, bufs=1)
            S.activation(dtv[:], e1[:], AF.Ln, bias=onecolF[0:4, 0:1])
            ldt = T([4, 512], "mb_ldt", F32, bufs=1)
            S.activation(ldt[:], dtv[:], AF.Ln)
            dtA = T([4, 512], "mb_dtA", F32, bufs=1)
            V.tensor_scalar(dtA[:], dtv[:], eA_spa[:, i:i + 1], -1.0,
                            op0=ALU.mult, op1=ALU.mult)
            acum = T([4, 512], "mb_acum", F32, bufs=1)
            for s in range(BPC):
                V.tensor_tensor_scan(acum[:, s * 256:(s + 1) * 256],
                                     dtA[:, s * 256:(s + 1) * 256],
                                     dtA[:, s * 256:(s + 1) * 256], 0.0,
                                     op0=ALU.add, op1=ALU.bypass)
            aflat = T([1, 2, 1024], "aflat", F32, bufs=1)
            for s in range(BPC):
                dma(aflat[0:1, s, :].rearrange("o (p f) -> o p f", p=4),
                    acum[:, s * 256:(s + 1) * 256])
            ynt = T([128, 2, 2, 256], "mb_ynt")     # [p, j, s, t]
            for s in range(BPC):
                # acum+dt transposed: [tok, st, 8] f32 (cols 0:4 acum, 4:8 dt)
                acdtT = T([128, 2, 8], "spa_acdtT", F32)
                for st in range(2):
                    csl = slice(s * 256 + st * 128, s * 256 + (st + 1) * 128)
                    ptr = PB((128, 8))
                    TR(ptr[:, 0:4], acum[:, csl], identF[0:4, 0:4])
                    TR(ptr[:, 4:8], ldt[:, csl], identF[0:4, 0:4])
                    S.copy(acdtT[:, st, :], ptr[:, 0:8])
                pb1 = PB()
                MM(pb1[:], onesF[:], aflat[:, s, 0:512], start=True, stop=True)
                pb2 = PB()
                MM(pb2[:], onesF[:], aflat[:, s, 512:1024], start=True, stop=True)
                Dt = T([128, 2, 4, 256], "ssd_Dt")
                for st in range(2):
                    for h in range(H1):
                        pbx = pb1 if h < 2 else pb2
                        V.scalar_tensor_tensor(
                            Dt[:, st, h, :], pbx[:, (h % 2) * 256:(h % 2 + 1) * 256],
                            acdtT[:, st, h:h + 1], ct['minf_spa'][:, st, :],
                            op0=ALU.subtract, op1=ALU.min)
                # Et = dt * exp(Dt) via ln(dt) bias
                Et = T([128, 2, 4, 256], "ssd_Et")
                for st in range(2):
                    for h in range(H1):
                        S.activation(Et[:, st, h, :], Dt[:, st, h, :], AF.Exp,
                                     bias=acdtT[:, st, 4 + h:5 + h])
                pm0s = T([128, 2, 256], "pm0s")
                for st in range(2):
                    pm0 = PB((128, 256))
                    MM(pm0[:], xcB[:, s, st * 128:(st + 1) * 128],
                       xcC[:, s, :], start=True, stop=True)
                    S.copy(pm0s[:, st, :], pm0[:])
                MT = T([128, 2, 4, 256], "ssd_MT")
                for st in range(2):
                    for h in range(H1):
                        V.tensor_tensor(MT[:, st, h, :], Et[:, st, h, :],
                                        pm0s[:, st, :], op=ALU.mult)
                # token-major xc
                xtm = T([128, 2, 2, 128], "spa_xtm")   # [tok, st, j, 128]
                for st in range(2):
                    for j in range(2):
                        ptr = PB((128, 128), BF16)
                        TR(ptr[:, 0:128],
                           xcx[:, j, s, st * 128:(st + 1) * 128], ident[:])
                        S.copy(xtm[:, st, j, :], ptr[:, 0:128])
                ypY = PA()
                for st in range(2):
                    for h in range(H1):
                        MM(ypY[(h % 2) * 64:(h % 2) * 64 + 64,
                               (h // 2) * 256:(h // 2 + 1) * 256],
                           xtm[:, st, h // 2, (h % 2) * 64:(h % 2) * 64 + 64],
                           MT[:, st, h, :],
                           start=(st == 0), stop=(st == 1),
                           tile_position=(0, (h % 2) * 64),
                           skip_group_check=True)
                ygt = T([128, 2, 256], "spa_ygt")
                for j in range(2):
                    y0 = T([128, 256], "spa_y0")
                    V.scalar_tensor_tensor(y0[:], xcx[:, j, s, :],
                                           col(f"spa_dpc{i}_{j}"),
                                           ypY[:, j * 256:(j + 1) * 256],
                                           op0=ALU.mult, op1=ALU.add)
                    V.tensor_tensor(ygt[:, j, :], y0[:],
                                    zsil[:, j, s * 256:(s + 1) * 256], op=ALU.mult)
                # gated RMS over d_inner (256)
                sqy = T([128, 2, 256], "mb_sqy")
                S.activation(sqy[:].rearrange("p j t -> p (j t)"),
                             ygt[:].rearrange("p j t -> p (j t)"), AF.Square)
                ssy = PS((1, 256))
                for j in range(2):
                    MM(ssy[:], onescolB, sqy[:, j, :],
                       start=(j == 0), stop=(j == 1))
                varr = T([1, 256], "rms_var", F32, bufs=1)
                V.tensor_scalar(varr[:], ssy[:], 1.0 / 256, EPS,
                                op0=ALU.mult, op1=ALU.add)
                rl = T([1, 256], "rms_rl", F32, bufs=1)
                S.activation(rl[:], varr[:], AF.Ln)
                rrow = T([1, 256], "rms_rrow", BF16)
                S.activation(rrow[:], rl[:], AF.Exp, scale=-0.5)
                rB = PB((128, 256))
                MM(rB[:], onesrowB, rrow[:], start=True, stop=True)
                for j in range(2):
                    V.scalar_tensor_tensor(ynt[:, j, s, :], ygt[:, j, :],
                                           col(f"spa_rwc{i}_{j}"),
                                           rB[:], op0=ALU.mult, op1=ALU.mult)
            pop = PA()
            for j in range(2):
                MM(pop[:].rearrange("p (s t) -> p s t", s=2),
                   wt['spa_out_pk'][:, i, j, :], ynt[:, j, :, :],
                   start=(j == 0), stop=(j == 1))
            h1 = T([128, 2, 256], "h1")
            V.tensor_tensor(h1[:].rearrange("p s t -> p (s t)"), pop[:],
                            xs[:].rearrange("p s t -> p (s t)"), op=ALU.add)
            return h1

        # ================= spe mamba =================
        def spe_mamba(i, h1):
            # LayerNorm over the 256 features (free dim), per sample
            mus = T([128, 2], "spe_mus", F32)
            V.tensor_reduce(mus[:], h1[:], axis=AX.X, op=ALU.add)
            sq2 = T([128, 512], "ln_sq")
            S.activation(sq2[:], h1[:].rearrange("p s t -> p (s t)"), AF.Square)
            ss2 = T([128, 2], "spe_ss2", F32)
            V.tensor_reduce(ss2[:], sq2[:].rearrange("p (s t) -> p s t", s=2),
                            axis=AX.X, op=ALU.add)
            mean = T([128, 2], "spe_mean", F32)
            V.tensor_scalar(mean[:], mus[:], 1.0 / 256, None, op0=ALU.mult)
            m2 = T([128, 2], "spe_m2", F32)
            V.tensor_tensor(m2[:], mean[:], mean[:], op=ALU.mult)
            var2 = T([128, 2], "spe_var", F32)
            V.scalar_tensor_tensor(var2[:], ss2[:], 1.0 / 256, m2[:],
                                   op0=ALU.mult, op1=ALU.subtract)
            l2t = T([128, 2], "spe_l2", F32)
            S.activation(l2t[:], var2[:], AF.Ln, bias=epscol[:, 0:1])
            rstd2 = T([128, 2], "spe_rstd", F32)
            S.activation(rstd2[:], l2t[:], AF.Exp, scale=-0.5)
            X2fp = T([128, 2, 2, 131], "x2fp")      # [feat, s, kc, 3+tok]
            V.memset(X2fp[:, :, :, 0:3], 0.0)
            for s in range(BPC):
                xn = T([128, 256], "spe_xn")
                V.tensor_scalar(xn[:], h1[:, s, :], mean[:, s:s + 1], rstd2[:, s:s + 1],
                                op0=ALU.subtract, op1=ALU.mult)
                u = T([128, 256], "spe_u")
                V.tensor_tensor(u[:], xn[:], wt['spe_ln_wB'][:, i, :], op=ALU.mult)
                xsn = T([128, 256], "spe_xsn")
                V.tensor_tensor(xsn[:], u[:], wt['spe_ln_bB'][:, i, :], op=ALU.add)
                for ft in range(2):
                    ptr = PB((128, 128), BF16)
                    TR(ptr[:, 0:128], xsn[:, ft * 128:(ft + 1) * 128], ident[:])
                    S.copy(X2fp[:, s, ft, 3:131], ptr[:, 0:128])
            zsil = T([128, 2, 2, 2, 128], "mb_zsil")   # [p, g, jj, s, t]
            for g in range(2):
                pz = PA()
                for jj in range(2):
                    for kc in range(2):
                        MM(pz[:].rearrange("p (a s t) -> p a s t", a=2, s=2)[:, jj, :, :],
                           wt['spe_z_w'][:, i, kc,
                                         (2 * g + jj) * 128:(2 * g + jj + 1) * 128],
                           X2fp[:, :, kc, 3:131],
                           start=(kc == 0), stop=(kc == 1), skip_group_check=True)
                S.activation(zsil[:, g, :, :, :].rearrange("p a s t -> p (a s t)"),
                             pz[:], AF.Silu)
            xc2 = T([128, 4, 2, 128], "mb_xcx")     # [p, j, s, t]
            xcB = T([64, 2, 128], "mb_xcB")
            xcC = T([64, 2, 128], "mb_xcC")
            blocks = ([(blk * 128, 128, xc2[:, blk, :, :], col(f"spe_cb{i}_{blk}"))
                       for blk in range(4)]
                      + [(512, 64, xcB[:], col(f"spe_cbB{i}", 64)),
                         (576, 64, xcC[:], col(f"spe_cbC{i}", 64))])
            for c0, rows, dst, cb in blocks:
                px = PA((rows, 256))
                for k in range(4):
                    for kc in range(2):
                        MM(px[:].rearrange("p (s t) -> p s t", s=2),
                           wt['spe_xc_w'][:, i, k, kc, c0:c0 + rows],
                           X2fp[:, :, kc, k:k + 128],
                           start=(k == 0 and kc == 0), stop=(k == 3 and kc == 1))
                S.activation(dst.rearrange("p s t -> p (s t)"), px[:],
                             AF.Silu, bias=cb)
            # dt path (fp32)
            pdt = PS((8, 256))
            for kc in range(2):
                MM(pdt[:], wt['spe_dt_w'][:, i, kc, :], X2fp[:, :, kc, 3:131],
                   start=(kc == 0), stop=(kc == 1))
            e1 = T([8, 256], "mb_e1", F32, bufs=1)
            S.activation(e1[:], pdt[:], AF.Exp, bias=col(f"spe_dtb{i}", 8))
            dtv = T([8, 256], "mb_dtv", F32, bufs=1)
            S.activation(dtv[:], e1[:], AF.Ln, bias=onecolF[0:8, 0:1])
            ldt = T([8, 256], "mb_ldt", F32, bufs=1)
            S.activation(ldt[:], dtv[:], AF.Ln)
            dtA = T([8, 256], "mb_dtA", F32, bufs=1)
            V.tensor_scalar(dtA[:], dtv[:], eA_spe[:, i:i + 1], -1.0,
                            op0=ALU.mult, op1=ALU.mult)
            acum = T([8, 256], "mb_acum", F32, bufs=1)
            for s in range(BPC):
                V.tensor_tensor_scan(acum[:, s * 128:(s + 1) * 128],
                                     dtA[:, s * 128:(s + 1) * 128],
                                     dtA[:, s * 128:(s + 1) * 128], 0.0,
                                     op0=ALU.add, op1=ALU.bypass)
            aflat = T([1, 2, 1024], "aflat", F32, bufs=1)
            for s in range(BPC):
                dma(aflat[0:1, s, :].rearrange("o (p f) -> o p f", p=8),
                    acum[:, s * 128:(s + 1) * 128])
            ynt = T([128, 4, 2, 128], "mb_ynt")     # [p, j, s, t]
            for s in range(BPC):
                acdtT = T([128, 16], "spe_acdtT", F32)
                csl = slice(s * 128, (s + 1) * 128)
                ptr = PB((128, 16))
                TR(ptr[:, 0:8], acum[:, csl], identF[0:8, 0:8])
                TR(ptr[:, 8:16], ldt[:, csl], identF[0:8, 0:8])
                S.copy(acdtT[:], ptr[:, 0:16])
                pb1 = PB()
                MM(pb1[:], onesF[:], aflat[:, s, 0:512], start=True, stop=True)
                pb2 = PB()
                MM(pb2[:], onesF[:], aflat[:, s, 512:1024], start=True, stop=True)
                Dt = T([128, 8, 128], "ssd_Dt")
                for h in range(H2):
                    pbx = pb1 if h < 4 else pb2
                    V.scalar_tensor_tensor(
                        Dt[:, h, :], pbx[:, (h % 4) * 128:(h % 4 + 1) * 128],
                        acdtT[:, h:h + 1], ct['minf_spe'][:],
                        op0=ALU.subtract, op1=ALU.min)
                Et = T([128, 8, 128], "ssd_Et")
                for h in range(H2):
                    S.activation(Et[:, h, :], Dt[:, h, :], AF.Exp,
                                 bias=acdtT[:, 8 + h:9 + h])
                pm0 = PB((128, 128))
                MM(pm0[:], xcB[:, s, :], xcC[:, s, :],
                   start=True, stop=True)
                pm0s = T([128, 128], "pm0s")
                S.copy(pm0s[:], pm0[:])
                MT = T([128, 8, 128], "ssd_MT")
                for h in range(H2):
                    V.tensor_tensor(MT[:, h, :], Et[:, h, :],
                                    pm0s[:], op=ALU.mult)
                xtm2 = T([128, 4, 128], "spa_xtm")  # [tok, j, 128]
                for j in range(4):
                    ptr = PB((128, 128), BF16)
                    TR(ptr[:, 0:128], xc2[:, j, s, :], ident[:])
                    S.copy(xtm2[:, j, :], ptr[:, 0:128])
                ypY = PA()
                for j in range(4):
                    for hh in range(2):
                        MM(ypY[hh * 64:hh * 64 + 64, j * 128:(j + 1) * 128],
                           xtm2[:, j, hh * 64:hh * 64 + 64],
                           MT[:, 2 * j + hh, :], start=True, stop=True,
                           tile_position=(0, hh * 64),
                           skip_group_check=True)
                ygt = T([128, 4, 128], "spe_ygt")
                for j in range(4):
                    y0 = T([128, 128], "spe_y0")
                    V.scalar_tensor_tensor(y0[:], xc2[:, j, s, :],
                                           col(f"spe_dpc{i}_{j}"),
                                           ypY[:, j * 128:(j + 1) * 128],
                                           op0=ALU.mult, op1=ALU.add)
                    V.tensor_tensor(ygt[:, j, :], y0[:],
                                    zsil[:, j // 2, j % 2, s, :], op=ALU.mult)
                sqy = T([128, 4, 128], "mb_sqy")
                S.activation(sqy[:].rearrange("p j t -> p (j t)"),
                             ygt[:].rearrange("p j t -> p (j t)"), AF.Square)
                ssy = PS((1, 128))
                for j in range(4):
                    MM(ssy[:], onescolB, sqy[:, j, :],
                       start=(j == 0), stop=(j == 3))
                varr = T([1, 128], "rms_var", F32, bufs=1)
                V.tensor_scalar(varr[:], ssy[:], 1.0 / 512, EPS,
                                op0=ALU.mult, op1=ALU.add)
                rl = T([1, 128], "rms_rl", F32, bufs=1)
                S.activation(rl[:], varr[:], AF.Ln)
                rrow = T([1, 128], "rms_rrow", BF16)
                S.activation(rrow[:], rl[:], AF.Exp, scale=-0.5)
                rB = PB((128, 128))
                MM(rB[:], onesrowB, rrow[:], start=True, stop=True)
                for j in range(4):
                    V.scalar_tensor_tensor(ynt[:, j, s, :], ygt[:, j, :],
                                           col(f"spe_rwc{i}_{j}"),
                                           rB[:], op0=ALU.mult, op1=ALU.mult)
            xs_new = T([128, 2, 256], "xs")
            for ft in range(2):
                ph2 = PB((128, 256))
                for k in range(4):
                    MM(ph2[:].rearrange("p (s t) -> p s t", s=2),
                       wt['spe_out_pk'][:, i, k, ft * 128:(ft + 1) * 128],
                       ynt[:, k, :, :], start=(k == 0), stop=(k == 3))
                h2f = T([128, 256], "spe_h2f")
                S.copy(h2f[:], ph2[:])
                for s in range(BPC):
                    ptr = PB((128, 128), BF16)
                    TR(ptr[:, 0:128], h2f[:, s * 128:(s + 1) * 128], ident[:])
                    V.tensor_tensor(xs_new[:, s, ft * 128:(ft + 1) * 128],
                                    ptr[:, 0:128], h1[:, s, ft * 128:(ft + 1) * 128],
                                    op=ALU.add)
            return xs_new

        # ================= layers =================
        cur = xs
        for i in range(2):
            h1 = spa_mamba(i, cur)
            tap(f"h1_{i}", h1[:].rearrange("p s t -> p (s t)"), (128, 512))
            cur = spe_mamba(i, h1)
            tap(f"xsl{i + 1}", cur[:].rearrange("p s t -> p (s t)"), (128, 512))

        # ================= final LN =================
        xf = T([128, 2, 256], "xf")
        part_ln(cur[:], 2, xf[:])
        xfl = xf[:].rearrange("p s t -> p (s t)")
        tap("xf", xfl, (128, 512))

        # ================= spa attention (center query) =================
        pctr = PS((128, 2))
        for l in range(5):
            MM(pctr[:], wt['cprj_pk'][:, l, :], xf[:, :, l],
               start=(l == 0), stop=(l == 4))
        ctr = T([128, 2], "at_ctr")
        S.activation(ctr[:], pctr[:], AF.Identity, bias=col("cprj_b"))
        pq = PS((128, 2))
        MM(pq[:], wt['aqT'][:], ctr[:], start=True, stop=True)
        qsb = T([128, 2], "at_q")
        S.activation(qsb[:], pq[:], AF.Identity, bias=col("aq_b"))
        pk = PA()
        MM(pk[:], wt['akT'][:], xfl, start=True, stop=True)
        Ksb = T([128, 2, 256], "at_K")
        S.activation(Ksb[:].rearrange("p s t -> p (s t)"), pk[:], AF.Identity,
                     bias=col("ak_b"))
        pv = PA()
        MM(pv[:], wt['avT'][:], xfl, start=True, stop=True)
        Vsb = T([128, 2, 256], "at_V")
        S.activation(Vsb[:].rearrange("p s t -> p (s t)"), pv[:], AF.Identity,
                     bias=col("av_b"))
        vo = T([128, 2, 256], "at_vo")
        for s in range(BPC):
            qd = T([128, 8], "at_qd")
            V.tensor_tensor(qd[:], qsb[:, s:s + 1].to_broadcast((128, 8)),
                            ct['Emask_q'][:], op=ALU.mult)
            plg = PS((8, 256))
            MM(plg[:], qd[:], Ksb[:, s, :], start=True, stop=True)
            nm = T([8, 1], "at_nm", F32)
            V.tensor_reduce(nm[:], plg[:], axis=AX.X, op=ALU.max, negate=True)
            nm4 = T([8, 1], "at_nm4", F32)
            V.tensor_scalar(nm4[:], nm[:], 0.25, None, op0=ALU.mult)
            ex = T([8, 256], "at_ex")
            S.activation(ex[:], plg[:], AF.Exp, bias=nm4[:, 0:1], scale=0.25)
            sm = T([8, 1], "at_sm", F32)
            V.tensor_reduce(sm[:], ex[:], axis=AX.X, op=ALU.add)
            rc = T([8, 1], "at_rc", F32)
            V.reciprocal(rc[:], sm[:])
            aw = T([8, 256], "at_aw")
            V.tensor_scalar(aw[:], ex[:], rc[:, 0:1], None, op0=ALU.mult)
            patB = PB((128, 256))
            MM(patB[:], ct['E_attn'][:], aw[:], start=True, stop=True)
            V.tensor_tensor(vo[:, s, :], Vsb[:, s, :], patB[:], op=ALU.mult)
        pao = PA()
        MM(pao[:], wt['aoT'][:], vo[:].rearrange("p s t -> p (s t)"),
           start=True, stop=True)
        xa = T([128, 2, 256], "xa")
        V.scalar_tensor_tensor(xa[:].rearrange("p s t -> p (s t)"), pao[:],
                               col("ao_b"), xfl, op0=ALU.add, op1=ALU.add)
        tap("xa", xa[:].rearrange("p s t -> p (s t)"), (128, 512))

        # ================= spe attention =================
        X2a = T([128, 2, 2, 128], "x2fp")
        for s in range(BPC):
            for ft in range(2):
                ptr = PB((128, 128), BF16)
                TR(ptr[:, 0:128], xa[:, s, ft * 128:(ft + 1) * 128], ident[:])
                S.copy(X2a[:, s, ft, :], ptr[:, 0:128])
        q2 = T([128, 2, 2, 128], "sp2_q2")   # [p, ot, s, t]
        k2 = T([128, 2, 2, 128], "sp2_k2")
        for ot in range(2):
            pq2 = PB((128, 256))
            for ft in range(2):
                MM(pq2[:].rearrange("p (s t) -> p s t", s=2),
                   wt['sqT'][:, ft, ot * 128:(ot + 1) * 128],
                   X2a[:, :, ft, :], start=(ft == 0), stop=(ft == 1))
            S.activation(q2[:, ot, :, :].rearrange("p s t -> p (s t)"),
                         pq2[:], AF.Identity, bias=col(f"sq_b{ot}"))
            pk2 = PB((128, 256))
            for ft in range(2):
                MM(pk2[:].rearrange("p (s t) -> p s t", s=2),
                   wt['skT'][:, ft, ot * 128:(ot + 1) * 128],
                   X2a[:, :, ft, :], start=(ft == 0), stop=(ft == 1))
            S.activation(k2[:, ot, :, :].rearrange("p s t -> p (s t)"),
                         pk2[:], AF.Identity, bias=col(f"sk_b{ot}"))
        xs2 = T([128, 2, 256], "xs2")
        for s in range(BPC):
            pv2 = PB((128, 256))
            for ft in range(2):
                MM(pv2[:], X2a[:, s, ft, :], wt['svT'][:, ft, :],
                   start=(ft == 0), stop=(ft == 1))
            v2 = T([128, 256], "sp2_v2")
            V.tensor_tensor(v2[:], pv2[:], wt['svbB'][:], op=ALU.add)
            pa2 = PB((128, 128))
            for ot in range(2):
                MM(pa2[:, 0:128], q2[:, ot, s, :], k2[:, ot, s, :],
                   start=(ot == 0), stop=(ot == 1))
            nm = T([128, 1], "sp2_nm", F32)
            V.tensor_reduce(nm[:], pa2[:, 0:128], axis=AX.X, op=ALU.max, negate=True)
            nm16 = T([128, 1], "sp2_nm16", F32)
            V.tensor_scalar(nm16[:], nm[:], 1.0 / 16, None, op0=ALU.mult)
            ex = T([128, 128], "sp2_ex")
            S.activation(ex[:], pa2[:, 0:128], AF.Exp, bias=nm16[:, 0:1], scale=1.0 / 16)
            sm = T([128, 1], "sp2_sm", F32)
            V.tensor_reduce(sm[:], ex[:], axis=AX.X, op=ALU.add)
            rc = T([128, 1], "sp2_rc", F32)
            V.reciprocal(rc[:], sm[:])
            a2 = T([128, 128], "sp2_a2")
            V.tensor_scalar(a2[:], ex[:], rc[:, 0:1], None, op0=ALU.mult)
            pa2T = PB((128, 128), BF16)
            TR(pa2T[:, 0:128], a2[:], ident[:])
            a2T = T([128, 128], "sp2_a2T")
            S.copy(a2T[:], pa2T[:, 0:128])
            o2 = T([128, 2, 128], "sp2_o2")
            for ot in range(2):
                po2 = PB((128, 128))
                MM(po2[:, 0:128], v2[:, ot * 128:(ot + 1) * 128], a2T[:],
                   start=True, stop=True)
                S.copy(o2[:, ot, :], po2[:, 0:128])
            po3 = PB((128, 256))
            for ot in range(2):
                MM(po3[:], o2[:, ot, :], wt['soT'][:, ot, :],
                   start=(ot == 0), stop=(ot == 1))
            t3 = T([128, 256], "sp2_t3")
            V.tensor_tensor(t3[:], po3[:], wt['sobB'][:], op=ALU.add)
            V.tensor_tensor(xs2[:, s, :], t3[:], xa[:, s, :], op=ALU.add)
        tap("xs2", xs2[:].rearrange("p s t -> p (s t)"), (128, 512))

        # ================= downsample =================
        invr = T([1, BPC, L], "irow_raw", I32, bufs=1)
        dma(invr[:], inv[None, :, :])
        invf = T([1, BPC, L], "irow_f", F32, bufs=1)
        V.tensor_copy(invf[:], invr[:])
        pds = psD.tile([64, 512], F32, tag="ds", name="pds")
        for s in range(BPC):
            invB = PB()
            MM(invB[:, 0:L], onesF[:], invf[:, s, :], start=True, stop=True)
            QT = T([128, 2, 256], "perm_oh")
            for tt in range(2):
                V.tensor_scalar(QT[:, tt, :], invB[:, 0:L], ct['iotaC'][:, tt:tt + 1],
                                None, op0=ALU.is_equal)
            tmv = T([128, 2, 128], "tm_tmp")
            for tt in range(2):
                ptr = PB((128, 128), BF16)
                TR(ptr[:, 0:128], xs2[:, s, tt * 128:(tt + 1) * 128], ident[:])
                S.copy(tmv[:, tt, :], ptr[:, 0:128])
            pxr = PB((128, 256))
            for tt in range(2):
                MM(pxr[:], tmv[:, tt, :], QT[:, tt, :],
                   start=(tt == 0), stop=(tt == 1))
            xrp = T([128, 324], "ds_xrp")
            V.memset(xrp[:], 0.0)
            xr3 = xrp[:].rearrange("p (h w) -> p h w", h=18)
            S.copy(xr3[:, 1:17, 1:17], pxr[:].rearrange("p (h w) -> p h w", h=16))
            for kh in range(3):
                for kw in range(3):
                    k = kh * 3 + kw
                    cmp_ = T([128, 64], "ds_cmp")
                    V.tensor_copy(cmp_[:].rearrange("p (a b) -> p a b", a=8),
                                  xr3[:, kh:kh + 16:2, kw:kw + 16:2])
                    MM(pds[:, s * 128:(s + 1) * 128],
                       cmp_[:],
                       wt['dsw_pk'][:, k, :],
                       start=(k == 0), stop=(k == 8),
                       skip_group_check=True)
        for s in range(BPC):
            view = pds[:, s * 128:(s + 1) * 128]
            mus = T([64, 1], "ds_mus", F32)
            V.tensor_reduce(mus[:], view, axis=AX.X, op=ALU.add)
            mean = T([64, 1], "ds_mean", F32)
            V.tensor_scalar(mean[:], mus[:], 1.0 / 128, None, op0=ALU.mult)
            sq = T([64, 128], "ds_sq", F32)
            S.activation(sq[:], view, AF.Square)
            ss = T([64, 1], "ds_ss", F32)
            V.tensor_reduce(ss[:], sq[:], axis=AX.X, op=ALU.add)
            m2 = T([64, 1], "ds_m2", F32)
            V.tensor_tensor(m2[:], mean[:], mean[:], op=ALU.mult)
            var = T([64, 1], "ds_var", F32)
            V.scalar_tensor_tensor(var[:], ss[:], 1.0 / 128, m2[:],
                                   op0=ALU.mult, op1=ALU.subtract)
            lv = T([64, 1], "ds_lv", F32)
            S.activation(lv[:], var[:], AF.Ln, bias=epscol[0:64, 0:1])
            rstd = T([64, 1], "ds_rstd", F32)
            S.activation(rstd[:], lv[:], AF.Exp, scale=-0.5)
            xn = T([64, 128], "ds_xn", F32)
            V.tensor_scalar(xn[:], view, mean[:, 0:1], rstd[:, 0:1],
                            op0=ALU.subtract, op1=ALU.mult)
            t1 = T([64, 128], "ds_t1", F32)
            V.tensor_tensor(t1[:], xn[:], wt['ds_ln_wB'][:], op=ALU.mult)
            o1 = T([64, 128], "ds_o1", F32)
            V.tensor_tensor(o1[:], t1[:], wt['ds_ln_bB'][:], op=ALU.add)
            dma(out[s].rearrange("h w c -> (h w) c"), o1[:])

        stk.close()
    return nc, tap_t


# ---------------------------------------------------------------------------
_CACHE = {}


def _get_program(taps=()):
    key = tuple(sorted(taps))
    if key not in _CACHE:
        _CACHE[key] = build_program(taps)
    return _CACHE[key]


def make_inmaps(inputs, taps=()):
    cst = host_constants()
    w = prep_weights(inputs)
    x = np.asarray(inputs['x'], np.float32).reshape(16, C, L).astype(NPBF)
    idx = np.asarray(inputs['sorted_index'], np.int32)
    inv = np.argsort(idx, axis=1, kind='stable').astype(np.int32)
    in_maps = []
    for c in range(NCORES):
        m = {}
        m.update({k: np.ascontiguousarray(v) for k, v in cst.items()})
        m.update({k: np.ascontiguousarray(v) for k, v in w.items()})
        sl = slice(c * BPC, (c + 1) * BPC)
        m['x2'] = np.ascontiguousarray(x[sl])
        m['idx'] = np.ascontiguousarray(idx[sl])
        m['inv'] = np.ascontiguousarray(inv[sl])
        in_maps.append(m)
    return in_maps


def run(inputs, taps=(), trace=False):
    nc, tap_t = _get_program(taps)
    in_maps = make_inmaps(inputs, taps)
    res = run_bass_kernel_spmd(nc, in_maps, list(range(NCORES)), trace=trace)
    outs = np.concatenate([np.asarray(r['out'], np.float32) for r in res.results],
                          axis=0)
    tapd = {}
    for name in taps:
        tapd[name] = [np.asarray(r.get('t_' + name), np.float32)
                      for r in res.results]
    return outs, tapd, res


def kernel(**inputs):
    outs, _, _ = run(inputs)
    return outs

